# revision 15
# baseline (speedup 1.0000x reference)
"""FLASH (ShareA, FFConvM) Trainium2 kernel — 8-core SPMD.

Strategy (per the sharding hint): shard the 16384-token sequence across the
8 NeuronCores (2048 tokens each, group-aligned: 8 groups of 256 per core).
Each core computes both FFConvM branches with a 128-token halo so that the
17-tap depthwise convs match the unsharded reference exactly; the global
linear-attention path AllReduces the [128, 1024] lin_kv / lin_ku summaries;
the final FFConvM's depthwise conv exchanges an 8-token boundary halo via a
small AllGather.

All matmuls run in bf16 on the tensor engine (fp32 matmul is 1/4 rate);
LayerNorm statistics, PSUM accumulation, and the residual path stay fp32.
Depthwise convs are evaluated on the PE as 17 PSUM-accumulated matmuls with
per-tap diagonal weight matrices (the identity tap is folded in to add the
conv residual for free). Layout transposes (token-major <-> channel-major)
ride the DMA XBAR transpose engine in 128x128 bf16 blocks.
"""
import sys

if '/opt/trn_rl_repo' not in sys.path:
    sys.path.insert(0, '/opt/trn_rl_repo')

import numpy as np

import concourse.bass as bass
import concourse.tile as tile
from concourse import bacc, mybir
from concourse.bass_utils import run_bass_kernel_spmd

F32 = mybir.dt.float32
BF16 = mybir.dt.bfloat16
AF = mybir.ActivationFunctionType
ALU = mybir.AluOpType

N, D, H2, DV, QK, GS, KER = 16384, 512, 2048, 1024, 128, 256, 17
NC = 8
T = N // NC            # 2048 own tokens
HALO = 128
T2 = T + 2 * HALO      # 2304 pre-activation tokens
CP = (KER - 1) // 2    # 8 (conv halo)
RG = [list(range(NC))]

NCH_H = H2 // 128      # 16 chunks for h
NCH_ALL = NCH_H + 1 + D // 128   # 16 h + 1 qk + 4 out = 21
NTT = T // 128         # 16 token tiles of own range
NLT = T2 // 128        # 18 LN tiles


def _build_kernel():
    nc = bacc.Bacc("TRN2", target_bir_lowering=False, debug=False,
                   num_devices=NC)

    # ---------------- I/O ----------------
    x_sh = nc.dram_tensor("x_sh", [T2 + 1, D], F32, kind="ExternalInput").ap()
    thw = nc.dram_tensor("thw", [D, H2], BF16, kind="ExternalInput").ap()
    thb = nc.dram_tensor("thb", [H2], F32, kind="ExternalInput").ap()
    qkw = nc.dram_tensor("qkw", [D, QK], BF16, kind="ExternalInput").ap()
    qkb = nc.dram_tensor("qkb", [QK], F32, kind="ExternalInput").ap()
    outw = nc.dram_tensor("outw", [DV, D], BF16, kind="ExternalInput").ap()
    outb = nc.dram_tensor("outb", [D], F32, kind="ExternalInput").ap()
    dd = nc.dram_tensor("dd", [NCH_ALL, KER, 128, 128], BF16,
                        kind="ExternalInput").ap()
    osg = nc.dram_tensor("osg", [4, QK], F32, kind="ExternalInput").ap()
    osb = nc.dram_tensor("osb", [4, QK], F32, kind="ExternalInput").ap()
    invn = nc.dram_tensor("invn", [128, 1], F32, kind="ExternalInput").ap()
    medge = nc.dram_tensor("medge", [128, 256], F32, kind="ExternalInput").ap()
    sel = nc.dram_tensor("sel", [128, 16], F32, kind="ExternalInput").ap()
    y = nc.dram_tensor("y", [T, D], F32, kind="ExternalOutput").ap()

    with tile.TileContext(nc) as tc:
        _emit(nc, tc, x_sh, thw, thb, qkw, qkb, outw, outb, dd, osg, osb,
              invn, medge, sel, y)
    nc.compile()
    return nc


def _emit(nc, tc, x_sh, thw, thb, qkw, qkb, outw, outb, dd, osg, osb,
          invn, medge, sel, y):
    from contextlib import ExitStack
    ctx = ExitStack()
    with ctx:
        consts = ctx.enter_context(tc.tile_pool(name="consts", bufs=1))
        mm_ps = ctx.enter_context(tc.tile_pool(name="mm_ps", bufs=3, space="PSUM"))
        sim_ps = ctx.enter_context(tc.tile_pool(name="sim_ps", bufs=1, space="PSUM"))
        att_ps = ctx.enter_context(tc.tile_pool(name="att_ps", bufs=4, space="PSUM"))
        dram = ctx.enter_context(tc.tile_pool(name="dram", bufs=1, space="DRAM"))

        # ------------- constants to SBUF -------------
        qkw_sb = consts.tile([128, D // 128, QK], BF16)
        nc.sync.dma_start(qkw_sb[:], qkw.rearrange("(o p) f -> p o f", p=128))
        outw_sb = consts.tile([128, DV // 128, D], BF16)
        nc.sync.dma_start(outw_sb[:], outw.rearrange("(o p) f -> p o f", p=128))
        thb_sb = consts.tile([128, H2 // 128], F32)
        nc.sync.dma_start(thb_sb[:], thb.rearrange("(o p) -> p o", p=128))
        qkb_sb = consts.tile([128, 1], F32)
        nc.sync.dma_start(qkb_sb[:], qkb.rearrange("(o p) -> p o", p=128))
        outb_sb = consts.tile([128, D // 128], F32)
        nc.sync.dma_start(outb_sb[:], outb.rearrange("(o p) -> p o", p=128))
        osg_sb = consts.tile([128, 4], F32)
        nc.sync.dma_start(osg_sb[:], osg.rearrange("m p -> p m"))
        osb_sb = consts.tile([128, 4], F32)
        nc.sync.dma_start(osb_sb[:], osb.rearrange("m p -> p m"))
        inv_sb = consts.tile([128, 1], F32)
        nc.sync.dma_start(inv_sb[:], invn)
        me_sb = consts.tile([128, 256], F32)
        nc.sync.dma_start(me_sb[:], medge)
        sel_sb = consts.tile([128, 16], F32)
        nc.sync.dma_start(sel_sb[:], sel)
        eps_sb = consts.tile([128, 1], F32)
        nc.vector.memset(eps_sb[:], 1e-5)

        # resident activations (whole-kernel lifetime)
        qs_cm = consts.tile([128, 4, T], BF16)       # [d, m, t']  m: qq,lq,qk_,lk
        lin_kT = consts.tile([128, NTT, QK], BF16)   # [t'%128, tt, d]
        vuT = consts.tile([128, NTT, H2], BF16)      # [t'%128, tt, c] (v|u)
        linkvu_bf = consts.tile([128, 2 * DV], BF16)  # [d, (kv|ku)]

        # ------------- Stage 1: LN + transpose -------------
        with tc.tile_pool(name="thwp", bufs=1) as thwp, \
             tc.tile_pool(name="ln", bufs=3) as lnp, \
             tc.tile_pool(name="xnt", bufs=1) as xntp:
            thw_sb = thwp.tile([128, D // 128, H2], BF16)
            nc.sync.dma_start(thw_sb[:],
                              thw.rearrange("(o p) f -> p o f", p=128))
            xnT = xntp.tile([128, D // 128, T2], BF16)   # [ci%128, ci_chunk, j]
            for tt in range(NLT):
                r0 = tt * 128
                nxt = lnp.tile([128, D], F32, tag="nxt")
                nc.sync.dma_start(nxt[:, 0:D // 2], x_sh[r0:r0 + 128, 0:D // 2])
                nc.sync.dma_start(nxt[:, D // 2:D],
                                  x_sh[r0 + 1:r0 + 129, D // 2:D])
                stats = lnp.tile([128, 6], F32, tag="st")
                nc.vector.bn_stats(stats[:], nxt[:])
                mv = lnp.tile([128, 2], F32, tag="mv")
                nc.vector.bn_aggr(mv[:], stats[:])
                std = lnp.tile([128, 1], F32, tag="sd")
                nc.scalar.activation(std[:], mv[:, 1:2], AF.Sqrt, bias=eps_sb[:])
                nc.vector.reciprocal(std[:], std[:])
                xn = lnp.tile([128, D], BF16, tag="xn")
                nc.vector.tensor_scalar(xn[:], nxt[:], mv[:, 0:1], std[:],
                                        ALU.subtract, ALU.mult)
                for cb in range(D // 128):
                    nc.sync.dma_start_transpose(
                        xnT[:, cb, r0:r0 + 128], xn[:, cb * 128:(cb + 1) * 128])

            # ------------- Stage 2+3: linears + convs  -------------
            with tc.tile_pool(name="pre", bufs=3) as prep, \
                 tc.tile_pool(name="ddp", bufs=2) as ddp, \
                 tc.tile_pool(name="hcm", bufs=3) as hcmp:
                # order: qk chunk first (frees the qk/attention path early)
                for ch in range(NCH_H + 1):
                    is_qk = (ch == 0)
                    wch = NCH_H if is_qk else ch - 1   # chunk id in dd
                    pre = prep.tile([128, T2], BF16, tag="pre")
                    for r in range(5):
                        j0 = r * 512
                        w = 512 if r < 4 else T2 - 2048
                        ps = mm_ps.tile([128, 512], F32, tag="mm")
                        for ci in range(D // 128):
                            wsrc = (qkw_sb[:, ci, :] if is_qk else
                                    thw_sb[:, ci,
                                           (wch) * 128:(wch + 1) * 128])
                            nc.tensor.matmul(ps[:, :w], wsrc,
                                             xnT[:, ci, j0:j0 + w],
                                             start=(ci == 0),
                                             stop=(ci == D // 128 - 1))
                        bias = (qkb_sb[:, 0:1] if is_qk else
                                thb_sb[:, wch:wch + 1])
                        nc.scalar.activation(pre[:, j0:j0 + w], ps[:, :w],
                                             AF.Silu, bias=bias)
                    # zero tokens outside the global sequence (edge cores)
                    nc.vector.tensor_mul(pre[:, 0:128], pre[:, 0:128],
                                         me_sb[:, 0:128])
                    nc.vector.tensor_mul(pre[:, T2 - 128:T2],
                                         pre[:, T2 - 128:T2],
                                         me_sb[:, 128:256])

                    dk = ddp.tile([128, KER, 128], BF16, tag="dk")
                    nc.sync.dma_start(dk[:], dd[wch].rearrange("k p f -> p k f"))
                    for q in range(4):
                        ps = mm_ps.tile([128, 512], F32, tag="mm")
                        for k in range(KER):
                            nc.tensor.matmul(
                                ps[:], dk[:, k, :],
                                pre[:, 120 + q * 512 + k:632 + q * 512 + k],
                                start=(k == 0), stop=(k == KER - 1))
                        if is_qk:
                            for m in range(4):
                                nc.vector.tensor_scalar(
                                    qs_cm[:, m, q * 512:(q + 1) * 512], ps[:],
                                    osg_sb[:, m:m + 1], osb_sb[:, m:m + 1],
                                    ALU.mult, ALU.add)
                        else:
                            hcm = hcmp.tile([128, 512], BF16, tag="hcm")
                            nc.scalar.activation(hcm[:], ps[:], AF.Copy)
                            for b in range(4):
                                tb = q * 4 + b
                                nc.sync.dma_start_transpose(
                                    vuT[:, tb, (wch) * 128:(wch + 1) * 128],
                                    hcm[:, b * 128:(b + 1) * 128])
                    if is_qk:
                        for tb in range(NTT):
                            nc.sync.dma_start_transpose(
                                lin_kT[:, tb, :],
                                qs_cm[:, 3, tb * 128:(tb + 1) * 128])

        # ------------- Stage 4: lin_kv / lin_ku summaries + AllReduce ----
        ar_in = dram.tile([128, 2 * DV], F32)
        ar_out = dram.tile([128, 2 * DV], F32, addr_space="Shared")
        with tc.tile_pool(name="lkv", bufs=1) as lkvp:
            linkvu = lkvp.tile([128, 2 * DV], F32)
            for es in range(4):
                ps = mm_ps.tile([128, 512], F32, tag="mm")
                for tb in range(NTT):
                    nc.tensor.matmul(ps[:], lin_kT[:, tb, :],
                                     vuT[:, tb, es * 512:(es + 1) * 512],
                                     start=(tb == 0), stop=(tb == NTT - 1))
                nc.vector.tensor_scalar_mul(linkvu[:, es * 512:(es + 1) * 512],
                                            ps[:], inv_sb[:, 0:1])
            nc.sync.dma_start(ar_in[:], linkvu[:])
        nc.gpsimd.collective_compute("AllReduce", ALU.add, replica_groups=RG,
                                     ins=[ar_in[:]], outs=[ar_out[:]])
        nc.gpsimd.dma_start(linkvu_bf[:], ar_out[:])   # casts f32 -> bf16

        h2p_ctx = tc.tile_pool(name="h2cm", bufs=1)
        h2p = h2p_ctx.__enter__()
        h2_cm = h2p.tile([128, D // 128, T + 2 * CP], BF16)  # [co, ch, z]
        outsbp_ctx = tc.tile_pool(name="outsb", bufs=1)
        outsbp = outsbp_ctx.__enter__()
        out_sb = outsbp.tile([128, NTT, DV], BF16)  # gated out, token-major

        # ------------- Stage 5: quadratic attention + gating -------------
        with tc.tile_pool(name="attn", bufs=3) as attnp, \
             tc.tile_pool(name="gat", bufs=3) as gatp:
            for g in range(T // GS):
                j0 = g * GS
                rr = attnp.tile([128, 2, GS], BF16, tag="rr")
                for jh in range(2):
                    sps = sim_ps.tile([128, GS], F32, tag="sim")
                    nc.tensor.matmul(
                        sps[:], qs_cm[:, 2, j0 + jh * 128:j0 + (jh + 1) * 128],
                        qs_cm[:, 0, j0:j0 + GS], start=True, stop=True)
                    nc.scalar.activation(rr[:, jh, :], sps[:], AF.Relu)
                at = attnp.tile([128, 2, GS], BF16, tag="at")
                nc.vector.tensor_mul(at[:], rr[:], rr[:])
                for ih in range(2):
                    tt = 2 * g + ih
                    psl = []
                    for es in range(4):
                        ps = att_ps.tile([128, 512], F32, tag="att")
                        psl.append(ps)
                        for jh in range(2):
                            nc.tensor.matmul(
                                ps[:], at[:, jh, ih * 128:(ih + 1) * 128],
                                vuT[:, 2 * g + jh, es * 512:(es + 1) * 512],
                                start=(jh == 0), stop=False)
                        nc.tensor.matmul(
                            ps[:], qs_cm[:, 1, j0 + ih * 128:j0 + (ih + 1) * 128],
                            linkvu_bf[:, es * 512:(es + 1) * 512],
                            start=False, stop=True)
                    # gating: out = att_u * v * sigmoid(att_v * u)
                    t1 = gatp.tile([128, DV], BF16, tag="t1")
                    t2 = gatp.tile([128, DV], BF16, tag="t2")
                    for es in range(2):
                        sl = slice(es * 512, (es + 1) * 512)
                        nc.vector.tensor_mul(t1[:, sl], psl[es][:],
                                             vuT[:, tt, DV + es * 512:
                                                 DV + (es + 1) * 512])
                        nc.vector.tensor_mul(t2[:, sl], psl[es + 2][:],
                                             vuT[:, tt, sl])
                    sg = gatp.tile([128, DV], BF16, tag="sg")
                    nc.scalar.activation(sg[:], t1[:], AF.Sigmoid)
                    nc.vector.tensor_mul(out_sb[:, tt, :], t2[:], sg[:])

        # ------------- Stage 6: out-LN + out-linear -------------
        with tc.tile_pool(name="oln", bufs=3) as olnp, \
             tc.tile_pool(name="lnt", bufs=2) as lntp:
            for q in range(4):
                lnoT = lntp.tile([128, DV // 128, 512], BF16, tag="lnoT")
                for it in range(4):
                    tt = q * 4 + it
                    stats = olnp.tile([128, 2, 6], F32, tag="st")
                    nc.vector.bn_stats(stats[:, 0, :], out_sb[:, tt, 0:512])
                    nc.vector.bn_stats(stats[:, 1, :], out_sb[:, tt, 512:DV])
                    mv = olnp.tile([128, 2], F32, tag="mv")
                    nc.vector.bn_aggr(mv[:], stats[:])
                    std = olnp.tile([128, 1], F32, tag="sd")
                    nc.scalar.activation(std[:], mv[:, 1:2], AF.Sqrt,
                                         bias=eps_sb[:])
                    nc.vector.reciprocal(std[:], std[:])
                    lno = olnp.tile([128, DV], BF16, tag="lno")
                    nc.vector.tensor_scalar(lno[:], out_sb[:, tt, :],
                                            mv[:, 0:1], std[:],
                                            ALU.subtract, ALU.mult)
                    for cb in range(DV // 128):
                        nc.sync.dma_start_transpose(
                            lnoT[:, cb, it * 128:(it + 1) * 128],
                            lno[:, cb * 128:(cb + 1) * 128])
                for co in range(D // 128):
                    ps = mm_ps.tile([128, 512], F32, tag="mm")
                    for ci in range(DV // 128):
                        nc.tensor.matmul(ps[:],
                                         outw_sb[:, ci, co * 128:(co + 1) * 128],
                                         lnoT[:, ci, :],
                                         start=(ci == 0),
                                         stop=(ci == DV // 128 - 1))
                    nc.scalar.activation(
                        h2_cm[:, co, CP + q * 512:CP + (q + 1) * 512], ps[:],
                        AF.Silu, bias=outb_sb[:, co:co + 1])

        outsbp_ctx.__exit__(None, None, None)

        # ------------- Stage 7: AllGather conv halo -------------
        ag_in = dram.tile([16, D], F32)
        ag_out = dram.tile([NC * 16, D], F32, addr_space="Shared")
        for co in range(D // 128):
            cs = slice(co * 128, (co + 1) * 128)
            nc.gpsimd.dma_start(ag_in[0:8, cs].rearrange("t c -> c t"),
                                h2_cm[:, co, CP:CP + 8])
            nc.gpsimd.dma_start(ag_in[8:16, cs].rearrange("t c -> c t"),
                                h2_cm[:, co, T:T + CP])
        nc.gpsimd.collective_compute("AllGather", ALU.bypass, replica_groups=RG,
                                     ins=[ag_in[:]], outs=[ag_out[:]])
        with tc.tile_pool(name="agp", bufs=1) as agp:
            ag_sb = agp.tile([128, D], F32)
            nc.sync.dma_start(ag_sb[:], ag_out[:])
            for co in range(D // 128):
                hps = att_ps.tile([128, 512], F32, tag="att")
                nc.tensor.matmul(hps[:, 0:16], ag_sb[:, co * 128:(co + 1) * 128],
                                 sel_sb[:], start=True, stop=True)
                nc.vector.tensor_copy(h2_cm[:, co, 0:CP], hps[:, 0:CP])
                nc.vector.tensor_copy(h2_cm[:, co, T + CP:T + 2 * CP],
                                      hps[:, CP:2 * CP])

        # ------------- Stage 8: final conv + residual + store -------------
        with tc.tile_pool(name="h2f", bufs=1) as h2fp, \
             tc.tile_pool(name="fddp", bufs=2) as fddp, \
             tc.tile_pool(name="fcm", bufs=3) as fcmp:
            h2f_tm = h2fp.tile([128, NTT, D], BF16)  # h2+conv, token-major
            for co in range(D // 128):
                dk = fddp.tile([128, KER, 128], BF16, tag="fdk")
                nc.sync.dma_start(
                    dk[:], dd[NCH_H + 1 + co].rearrange("k p f -> p k f"))
                for q in range(4):
                    ps = mm_ps.tile([128, 512], F32, tag="mm")
                    for k in range(KER):
                        nc.tensor.matmul(ps[:], dk[:, k, :],
                                         h2_cm[:, co, q * 512 + k:
                                               q * 512 + k + 512],
                                         start=(k == 0), stop=(k == KER - 1))
                    fcm = fcmp.tile([128, 512], BF16, tag="fcm")
                    nc.scalar.activation(fcm[:], ps[:], AF.Copy)
                    for b in range(4):
                        tb = q * 4 + b
                        nc.sync.dma_start_transpose(
                            h2f_tm[:, tb, co * 128:(co + 1) * 128],
                            fcm[:, b * 128:(b + 1) * 128])
            with tc.tile_pool(name="fin", bufs=3) as finp:
                for tb in range(NTT):
                    xres = finp.tile([128, D], F32, tag="xr")
                    nc.sync.dma_start(xres[:],
                                      x_sh[129 + tb * 128:257 + tb * 128, :])
                    fin = finp.tile([128, D], F32, tag="fin")
                    nc.vector.tensor_add(fin[:], h2f_tm[:, tb, :], xres[:])
                    nc.sync.dma_start(y[tb * 128:(tb + 1) * 128, :], fin[:])
        h2p_ctx.__exit__(None, None, None)


_NC_CACHE = None


def _get_nc():
    global _NC_CACHE
    if _NC_CACHE is None:
        _NC_CACHE = _build_kernel()
    return _NC_CACHE


def _prep_inputs(inputs):
    """Host-side preprocessing: LN-affine folds, diag conv matrices,
    per-core shards."""
    g = {k: np.asarray(v) for k, v in inputs.items()}
    x = g['x'].reshape(N, D).astype(np.float32)
    inv_n = np.float32(g['inv_n'])

    thw = (g['th_ln_g'][:, None] * g['th_w']).astype(np.float32)
    thb = (g['th_b'] + g['th_ln_b'] @ g['th_w']).astype(np.float32)
    qkw = (g['qk_ln_g'][:, None] * g['qk_w']).astype(np.float32)
    qkb = (g['qk_b'] + g['qk_ln_b'] @ g['qk_w']).astype(np.float32)
    outw = (g['out_ln_g'][:, None] * g['out_w']).astype(np.float32)
    outb = (g['out_b'] + g['out_ln_b'] @ g['out_w']).astype(np.float32)
    osg = g['os_gamma'].astype(np.float32).copy()
    osb = g['os_beta'].astype(np.float32).copy()
    osg[0] /= GS
    osb[0] /= GS

    # diag conv matrices (identity tap folded in: +I at k=8)
    ddm = np.zeros((NCH_ALL, KER, 128, 128), np.float32)
    kers = [g['th_conv'][:, 0, :], g['qk_conv'][:, 0, :], g['out_conv'][:, 0, :]]
    chunk = 0
    for ker in kers:
        C = ker.shape[0]
        for cb in range(C // 128):
            for k in range(KER):
                v = ker[cb * 128:(cb + 1) * 128, k].copy()
                if k == CP:
                    v = v + 1.0
                np.fill_diagonal(ddm[chunk, k], v)
            chunk += 1
    assert chunk == NCH_ALL

    xpad = np.zeros((N + 2 * HALO + 1, D), np.float32)
    xpad[HALO + 1:HALO + 1 + N] = x

    import ml_dtypes
    bf = ml_dtypes.bfloat16
    shared = dict(
        thw=thw.astype(bf), thb=thb,
        qkw=qkw.astype(bf), qkb=qkb,
        outw=outw.astype(bf), outb=outb,
        dd=ddm.astype(bf),
        osg=osg, osb=osb,
        invn=np.full((128, 1), inv_n, np.float32),
    )

    in_maps = []
    for c in range(NC):
        s = c * T
        me = np.ones((128, 256), np.float32)
        if c == 0:
            me[:, :128] = 0.0
        if c == NC - 1:
            me[:, 128:] = 0.0
        selm = np.zeros((128, 16), np.float32)
        for m in range(8):
            if c > 0:
                selm[(c - 1) * 16 + 8 + m, m] = 1.0
            if c < NC - 1:
                selm[(c + 1) * 16 + m, 8 + m] = 1.0
        im = dict(shared)
        im['x_sh'] = np.ascontiguousarray(xpad[s:s + T2 + 1])
        im['medge'] = me
        im['sel'] = selm
        in_maps.append(im)
    return in_maps


def kernel(**inputs):
    nc = _get_nc()
    in_maps = _prep_inputs(inputs)
    res = run_bass_kernel_spmd(nc, in_maps, core_ids=list(range(NC)))
    out = np.concatenate([r['y'] for r in res.results], axis=0)
    return out.reshape(1, N, D).astype(np.float32)


# revision 26
# speedup vs baseline: 1.0201x; 1.0201x over previous
"""FLASH (ShareA, FFConvM) Trainium2 kernel — 8-core SPMD.

Strategy (per the sharding hint): shard the 16384-token sequence across the
8 NeuronCores (2048 tokens each, group-aligned: 8 groups of 256 per core).
Each core computes both FFConvM branches with a 128-token halo so that the
17-tap depthwise convs match the unsharded reference exactly; the global
linear-attention path AllReduces the [128, 1024] lin_kv / lin_ku summaries;
the final FFConvM's depthwise conv exchanges an 8-token boundary halo via a
small AllGather.

All matmuls run in bf16 on the tensor engine (fp32 matmul is 1/4 rate);
LayerNorm statistics, PSUM accumulation, and the residual path stay fp32.
Depthwise convs are evaluated on the PE as 17 PSUM-accumulated matmuls with
per-tap diagonal weight matrices (the identity tap is folded in to add the
conv residual for free). Layout transposes (token-major <-> channel-major)
ride the DMA XBAR transpose engine in 128x128 bf16 blocks.
"""
import sys

if '/opt/trn_rl_repo' not in sys.path:
    sys.path.insert(0, '/opt/trn_rl_repo')

import numpy as np

import concourse.bass as bass
import concourse.tile as tile
from concourse import bacc, mybir
from concourse.bass_utils import run_bass_kernel_spmd

F32 = mybir.dt.float32
BF16 = mybir.dt.bfloat16
AF = mybir.ActivationFunctionType
ALU = mybir.AluOpType

N, D, H2, DV, QK, GS, KER = 16384, 512, 2048, 1024, 128, 256, 17
NC = 8
T = N // NC            # 2048 own tokens
HALO = 128
T2 = T + 2 * HALO      # 2304 pre-activation tokens
CP = (KER - 1) // 2    # 8 (conv halo)
RG = [list(range(NC))]

NCH_H = H2 // 128      # 16 chunks for h
NCH_ALL = NCH_H + 1 + D // 128   # 16 h + 1 qk + 4 out = 21
NTT = T // 128         # 16 token tiles of own range
NLT = T2 // 128        # 18 LN tiles


def _build_kernel():
    nc = bacc.Bacc("TRN2", target_bir_lowering=False, debug=False,
                   num_devices=NC)

    # ---------------- I/O ----------------
    x_sh = nc.dram_tensor("x_sh", [T2 + 1, D], F32, kind="ExternalInput").ap()
    thw = nc.dram_tensor("thw", [D, H2], BF16, kind="ExternalInput").ap()
    thb = nc.dram_tensor("thb", [H2], F32, kind="ExternalInput").ap()
    qkw = nc.dram_tensor("qkw", [D, QK], BF16, kind="ExternalInput").ap()
    qkb = nc.dram_tensor("qkb", [QK], F32, kind="ExternalInput").ap()
    outw = nc.dram_tensor("outw", [DV, D], BF16, kind="ExternalInput").ap()
    outb = nc.dram_tensor("outb", [D], F32, kind="ExternalInput").ap()
    dd = nc.dram_tensor("dd", [NCH_ALL, KER, 128, 128], BF16,
                        kind="ExternalInput").ap()
    osg = nc.dram_tensor("osg", [4, QK], F32, kind="ExternalInput").ap()
    osb = nc.dram_tensor("osb", [4, QK], F32, kind="ExternalInput").ap()
    invn = nc.dram_tensor("invn", [128, 1], F32, kind="ExternalInput").ap()
    medge = nc.dram_tensor("medge", [128, 256], F32, kind="ExternalInput").ap()
    sel = nc.dram_tensor("sel", [128, 16], F32, kind="ExternalInput").ap()
    kerv = nc.dram_tensor("kerv", [NCH_ALL, 128, KER], F32,
                          kind="ExternalInput").ap()
    y = nc.dram_tensor("y", [T, D], F32, kind="ExternalOutput").ap()

    with tile.TileContext(nc) as tc:
        _emit(nc, tc, x_sh, thw, thb, qkw, qkb, outw, outb, dd, osg, osb,
              invn, medge, sel, kerv, y)
    nc.compile()
    return nc


DVE_TAPS = (2, 6, 10, 14)  # even shifts: bf16 2x mode stays aligned
GPS_TAPS = ()              # gpsimd lacks TensorScalarPtr on TRN2
PE_TAPS = tuple(k for k in range(KER) if k not in DVE_TAPS + GPS_TAPS)


def _emit(nc, tc, x_sh, thw, thb, qkw, qkb, outw, outb, dd, osg, osb,
          invn, medge, sel, kerv, y):
    from contextlib import ExitStack

    # XBAR transposes stay on the SP (sync) queue — the xbar path is tied
    # to it; plain bulk loads go to the Activation DGE queue so they don't
    # interleave (and mode-switch serialize) with the transposes.
    def dma_eng():
        return nc.scalar

    def dma_t(out, in_):
        nc.sync.dma_start_transpose(out, in_)

    ctx = ExitStack()
    with ctx:
        consts = ctx.enter_context(tc.tile_pool(name="consts", bufs=1))
        mm_ps = ctx.enter_context(tc.tile_pool(name="mm_ps", bufs=3, space="PSUM"))
        sim_ps = ctx.enter_context(tc.tile_pool(name="sim_ps", bufs=1, space="PSUM"))
        att_ps = ctx.enter_context(tc.tile_pool(name="att_ps", bufs=4, space="PSUM"))
        dram = ctx.enter_context(tc.tile_pool(name="dram", bufs=1, space="DRAM"))

        # ------------- constants to SBUF -------------
        qkw_sb = consts.tile([128, D // 128, QK], BF16)
        nc.sync.dma_start(qkw_sb[:], qkw.rearrange("(o p) f -> p o f", p=128))
        outw_sb = consts.tile([128, DV // 128, D], BF16)
        nc.sync.dma_start(outw_sb[:], outw.rearrange("(o p) f -> p o f", p=128))
        thb_sb = consts.tile([128, H2 // 128], F32)
        nc.sync.dma_start(thb_sb[:], thb.rearrange("(o p) -> p o", p=128))
        qkb_sb = consts.tile([128, 1], F32)
        nc.sync.dma_start(qkb_sb[:], qkb.rearrange("(o p) -> p o", p=128))
        outb_sb = consts.tile([128, D // 128], F32)
        nc.sync.dma_start(outb_sb[:], outb.rearrange("(o p) -> p o", p=128))
        osg_sb = consts.tile([128, 4], F32)
        nc.sync.dma_start(osg_sb[:], osg.rearrange("m p -> p m"))
        osb_sb = consts.tile([128, 4], F32)
        nc.sync.dma_start(osb_sb[:], osb.rearrange("m p -> p m"))
        inv_sb = consts.tile([128, 1], F32)
        nc.sync.dma_start(inv_sb[:], invn)
        me_sb = consts.tile([128, 256], F32)
        nc.sync.dma_start(me_sb[:], medge)
        sel_sb = consts.tile([128, 16], F32)
        nc.sync.dma_start(sel_sb[:], sel)
        kerv_sb = consts.tile([128, NCH_ALL, KER], F32)
        nc.sync.dma_start(kerv_sb[:], kerv.rearrange("c p k -> p c k"))
        eps_sb = consts.tile([128, 1], F32)
        nc.vector.memset(eps_sb[:], 1e-5)

        # resident activations (whole-kernel lifetime)
        qs_cm = consts.tile([128, 4, T], BF16)       # [d, m, t']  m: qq,lq,qk_,lk
        lin_kT = consts.tile([128, NTT, QK], BF16)   # [t'%128, tt, d]
        vuT = consts.tile([128, NTT, H2], BF16)      # [t'%128, tt, c] (v|u)
        linkvu_bf = consts.tile([128, 2 * DV], BF16)  # [d, (kv|ku)]

        # ------------- Stage 1: LN + transpose -------------
        with tc.tile_pool(name="thwp", bufs=1) as thwp, \
             tc.tile_pool(name="ln", bufs=3) as lnp, \
             tc.tile_pool(name="xnt", bufs=1) as xntp:
            thw_sb = thwp.tile([128, D // 128, H2], BF16)
            nc.sync.dma_start(thw_sb[:],
                              thw.rearrange("(o p) f -> p o f", p=128))
            xnT = xntp.tile([128, D // 128, T2], BF16)   # [ci%128, ci_chunk, j]
            for tt in range(NLT):
                r0 = tt * 128
                nxt = lnp.tile([128, D], F32, tag="nxt")
                dma_eng().dma_start(nxt[:, 0:D // 2], x_sh[r0:r0 + 128, 0:D // 2])
                dma_eng().dma_start(nxt[:, D // 2:D],
                                  x_sh[r0 + 1:r0 + 129, D // 2:D])
                stats = lnp.tile([128, 6], F32, tag="st")
                nc.vector.bn_stats(stats[:], nxt[:])
                mv = lnp.tile([128, 2], F32, tag="mv")
                nc.vector.bn_aggr(mv[:], stats[:])
                std = lnp.tile([128, 1], F32, tag="sd")
                nc.scalar.activation(std[:], mv[:, 1:2], AF.Sqrt, bias=eps_sb[:])
                nc.vector.reciprocal(std[:], std[:])
                xn = lnp.tile([128, D], BF16, tag="xn")
                nc.vector.tensor_scalar(xn[:], nxt[:], mv[:, 0:1], std[:],
                                        ALU.subtract, ALU.mult)
                for cb in range(D // 128):
                    dma_t(
                        xnT[:, cb, r0:r0 + 128], xn[:, cb * 128:(cb + 1) * 128])

            # ------------- Stage 2+3: linears + convs  -------------
            with tc.tile_pool(name="pre", bufs=3) as prep, \
                 tc.tile_pool(name="ddp", bufs=2) as ddp, \
                 tc.tile_pool(name="hcm", bufs=3) as hcmp:
                # order: qk chunk first (frees the qk/attention path early)
                for ch in range(NCH_H + 1):
                    is_qk = (ch == 0)
                    wch = NCH_H if is_qk else ch - 1   # chunk id in dd
                    pre = prep.tile([128, T2], BF16, tag="pre")
                    for r in range(5):
                        j0 = r * 512
                        w = 512 if r < 4 else T2 - 2048
                        ps = mm_ps.tile([128, 512], F32, tag="mm")
                        for ci in range(D // 128):
                            wsrc = (qkw_sb[:, ci, :] if is_qk else
                                    thw_sb[:, ci,
                                           (wch) * 128:(wch + 1) * 128])
                            nc.tensor.matmul(ps[:, :w], wsrc,
                                             xnT[:, ci, j0:j0 + w],
                                             start=(ci == 0),
                                             stop=(ci == D // 128 - 1))
                        bias = (qkb_sb[:, 0:1] if is_qk else
                                thb_sb[:, wch:wch + 1])
                        nc.scalar.activation(pre[:, j0:j0 + w], ps[:, :w],
                                             AF.Silu, bias=bias)
                    # zero tokens outside the global sequence (edge cores)
                    nc.vector.tensor_mul(pre[:, 0:128], pre[:, 0:128],
                                         me_sb[:, 0:128])
                    nc.vector.tensor_mul(pre[:, T2 - 128:T2],
                                         pre[:, T2 - 128:T2],
                                         me_sb[:, 128:256])

                    dk = ddp.tile([128, KER, 128], BF16, tag="dk")
                    dma_eng().dma_start(dk[:], dd[wch].rearrange("k p f -> p k f"))
                    pe_taps = range(KER) if is_qk else PE_TAPS
                    for q in range(4):
                        ps = mm_ps.tile([128, 512], F32, tag="mm")
                        for i, k in enumerate(pe_taps):
                            nc.tensor.matmul(
                                ps[:], dk[:, k, :],
                                pre[:, 120 + q * 512 + k:632 + q * 512 + k],
                                start=(i == 0), stop=(i == len(pe_taps) - 1))
                        if is_qk:
                            for m in range(4):
                                nc.vector.tensor_scalar(
                                    qs_cm[:, m, q * 512:(q + 1) * 512], ps[:],
                                    osg_sb[:, m:m + 1], osb_sb[:, m:m + 1],
                                    ALU.mult, ALU.add)
                        else:
                            hcm = hcmp.tile([128, 512], BF16, tag="hcm")
                            nc.scalar.activation(hcm[:], ps[:], AF.Copy)
                            # remaining taps on DVE / GpSimd (in-place FMA)
                            for k in DVE_TAPS:
                                nc.vector.scalar_tensor_tensor(
                                    hcm[:],
                                    pre[:, 120 + q * 512 + k:632 + q * 512 + k],
                                    kerv_sb[:, wch, k:k + 1], hcm[:],
                                    ALU.mult, ALU.add)
                            for k in GPS_TAPS:
                                nc.gpsimd.scalar_tensor_tensor(
                                    hcm[:],
                                    pre[:, 120 + q * 512 + k:632 + q * 512 + k],
                                    kerv_sb[:, wch, k:k + 1], hcm[:],
                                    ALU.mult, ALU.add)
                            for b in range(4):
                                tb = q * 4 + b
                                dma_t(
                                    vuT[:, tb, (wch) * 128:(wch + 1) * 128],
                                    hcm[:, b * 128:(b + 1) * 128])
                    if is_qk:
                        for tb in range(NTT):
                            dma_t(
                                lin_kT[:, tb, :],
                                qs_cm[:, 3, tb * 128:(tb + 1) * 128])

        # ------------- Stage 4: lin_kv / lin_ku summaries + AllReduce ----
        # bf16 AllReduce: halves the wire bytes; the summaries only feed the
        # (tiny-magnitude) linear-attention path so bf16 is ample.
        ar_in = dram.tile([128, 2 * DV], BF16)
        ar_out = dram.tile([128, 2 * DV], BF16, addr_space="Shared")
        with tc.tile_pool(name="lkv", bufs=1) as lkvp:
            linkvu = lkvp.tile([128, 2 * DV], BF16)
            for es in range(4):
                ps = mm_ps.tile([128, 512], F32, tag="mm")
                for tb in range(NTT):
                    nc.tensor.matmul(ps[:], lin_kT[:, tb, :],
                                     vuT[:, tb, es * 512:(es + 1) * 512],
                                     start=(tb == 0), stop=(tb == NTT - 1))
                nc.vector.tensor_scalar_mul(linkvu[:, es * 512:(es + 1) * 512],
                                            ps[:], inv_sb[:, 0:1])
            nc.sync.dma_start(ar_in[:], linkvu[:])
        nc.gpsimd.collective_compute("AllReduce", ALU.add, replica_groups=RG,
                                     ins=[ar_in[:]], outs=[ar_out[:]])
        nc.sync.dma_start(linkvu_bf[:], ar_out[:])

        h2p_ctx = tc.tile_pool(name="h2cm", bufs=1)
        h2p = h2p_ctx.__enter__()
        h2_cm = h2p.tile([128, D // 128, T + 2 * CP], BF16)  # [co, ch, z]
        outsbp_ctx = tc.tile_pool(name="outsb", bufs=1)
        outsbp = outsbp_ctx.__enter__()
        out_sb = outsbp.tile([128, NTT, DV], BF16)  # gated out, token-major

        # ------------- Stage 5: quadratic attention + gating -------------
        with tc.tile_pool(name="attn", bufs=3) as attnp, \
             tc.tile_pool(name="gat", bufs=3) as gatp:
            for g in range(T // GS):
                j0 = g * GS
                rr = attnp.tile([128, 2, GS], BF16, tag="rr")
                for jh in range(2):
                    sps = sim_ps.tile([128, GS], F32, tag="sim")
                    nc.tensor.matmul(
                        sps[:], qs_cm[:, 2, j0 + jh * 128:j0 + (jh + 1) * 128],
                        qs_cm[:, 0, j0:j0 + GS], start=True, stop=True)
                    nc.scalar.activation(rr[:, jh, :], sps[:], AF.Relu)
                at = attnp.tile([128, 2, GS], BF16, tag="at")
                nc.vector.tensor_mul(at[:], rr[:], rr[:])
                for ih in range(2):
                    tt = 2 * g + ih
                    psl = []
                    for es in range(4):
                        ps = att_ps.tile([128, 512], F32, tag="att")
                        psl.append(ps)
                        for jh in range(2):
                            nc.tensor.matmul(
                                ps[:], at[:, jh, ih * 128:(ih + 1) * 128],
                                vuT[:, 2 * g + jh, es * 512:(es + 1) * 512],
                                start=(jh == 0), stop=False)
                        nc.tensor.matmul(
                            ps[:], qs_cm[:, 1, j0 + ih * 128:j0 + (ih + 1) * 128],
                            linkvu_bf[:, es * 512:(es + 1) * 512],
                            start=False, stop=True)
                    # gating: out = att_u * v * sigmoid(att_v * u)
                    t1 = gatp.tile([128, DV], BF16, tag="t1")
                    t2 = gatp.tile([128, DV], BF16, tag="t2")
                    for es in range(2):
                        sl = slice(es * 512, (es + 1) * 512)
                        nc.vector.tensor_mul(t1[:, sl], psl[es][:],
                                             vuT[:, tt, DV + es * 512:
                                                 DV + (es + 1) * 512])
                        nc.vector.tensor_mul(t2[:, sl], psl[es + 2][:],
                                             vuT[:, tt, sl])
                    sg = gatp.tile([128, DV], BF16, tag="sg")
                    nc.scalar.activation(sg[:], t1[:], AF.Sigmoid)
                    nc.vector.tensor_mul(out_sb[:, tt, :], t2[:], sg[:])

        # ------------- Stage 6: out-LN + out-linear -------------
        with tc.tile_pool(name="oln", bufs=3) as olnp, \
             tc.tile_pool(name="lnt", bufs=2) as lntp:
            for q in range(4):
                lnoT = lntp.tile([128, DV // 128, 512], BF16, tag="lnoT")
                for it in range(4):
                    tt = q * 4 + it
                    stats = olnp.tile([128, 2, 6], F32, tag="st")
                    nc.vector.bn_stats(stats[:, 0, :], out_sb[:, tt, 0:512])
                    nc.vector.bn_stats(stats[:, 1, :], out_sb[:, tt, 512:DV])
                    mv = olnp.tile([128, 2], F32, tag="mv")
                    nc.vector.bn_aggr(mv[:], stats[:])
                    std = olnp.tile([128, 1], F32, tag="sd")
                    nc.scalar.activation(std[:], mv[:, 1:2], AF.Sqrt,
                                         bias=eps_sb[:])
                    nc.vector.reciprocal(std[:], std[:])
                    lno = olnp.tile([128, DV], BF16, tag="lno")
                    nc.vector.tensor_scalar(lno[:], out_sb[:, tt, :],
                                            mv[:, 0:1], std[:],
                                            ALU.subtract, ALU.mult)
                    for cb in range(DV // 128):
                        dma_t(
                            lnoT[:, cb, it * 128:(it + 1) * 128],
                            lno[:, cb * 128:(cb + 1) * 128])
                for co in range(D // 128):
                    ps = mm_ps.tile([128, 512], F32, tag="mm")
                    for ci in range(DV // 128):
                        nc.tensor.matmul(ps[:],
                                         outw_sb[:, ci, co * 128:(co + 1) * 128],
                                         lnoT[:, ci, :],
                                         start=(ci == 0),
                                         stop=(ci == DV // 128 - 1))
                    nc.scalar.activation(
                        h2_cm[:, co, CP + q * 512:CP + (q + 1) * 512], ps[:],
                        AF.Silu, bias=outb_sb[:, co:co + 1])

        outsbp_ctx.__exit__(None, None, None)

        # ------------- Stage 7: AllGather conv halo -------------
        ag_in = dram.tile([16, D], F32)
        ag_out = dram.tile([NC * 16, D], F32, addr_space="Shared")
        for co in range(D // 128):
            cs = slice(co * 128, (co + 1) * 128)
            nc.gpsimd.dma_start(ag_in[0:8, cs].rearrange("t c -> c t"),
                                h2_cm[:, co, CP:CP + 8])
            nc.gpsimd.dma_start(ag_in[8:16, cs].rearrange("t c -> c t"),
                                h2_cm[:, co, T:T + CP])
        nc.gpsimd.collective_compute("AllGather", ALU.bypass, replica_groups=RG,
                                     ins=[ag_in[:]], outs=[ag_out[:]])
        with tc.tile_pool(name="agp", bufs=1) as agp:
            ag_sb = agp.tile([128, D], F32)
            nc.sync.dma_start(ag_sb[:], ag_out[:])
            for co in range(D // 128):
                hps = att_ps.tile([128, 512], F32, tag="att")
                nc.tensor.matmul(hps[:, 0:16], ag_sb[:, co * 128:(co + 1) * 128],
                                 sel_sb[:], start=True, stop=True)
                nc.vector.tensor_copy(h2_cm[:, co, 0:CP], hps[:, 0:CP])
                nc.vector.tensor_copy(h2_cm[:, co, T + CP:T + 2 * CP],
                                      hps[:, CP:2 * CP])

        # ------------- Stage 8: final conv + residual + store -------------
        with tc.tile_pool(name="h2f", bufs=1) as h2fp, \
             tc.tile_pool(name="fddp", bufs=2) as fddp, \
             tc.tile_pool(name="fcm", bufs=3) as fcmp:
            h2f_tm = h2fp.tile([128, NTT, D], BF16)  # h2+conv, token-major
            for co in range(D // 128):
                dk = fddp.tile([128, KER, 128], BF16, tag="fdk")
                nc.sync.dma_start(
                    dk[:], dd[NCH_H + 1 + co].rearrange("k p f -> p k f"))
                # interior tiles (q=1,2) first: they don't need the AllGather
                # halo, so they overlap the collective's latency
                for q in (1, 2, 0, 3):
                    ps = mm_ps.tile([128, 512], F32, tag="mm")
                    for k in range(KER):
                        nc.tensor.matmul(ps[:], dk[:, k, :],
                                         h2_cm[:, co, q * 512 + k:
                                               q * 512 + k + 512],
                                         start=(k == 0), stop=(k == KER - 1))
                    fcm = fcmp.tile([128, 512], BF16, tag="fcm")
                    nc.scalar.activation(fcm[:], ps[:], AF.Copy)
                    for b in range(4):
                        tb = q * 4 + b
                        dma_t(
                            h2f_tm[:, tb, co * 128:(co + 1) * 128],
                            fcm[:, b * 128:(b + 1) * 128])
            with tc.tile_pool(name="fin", bufs=3) as finp:
                for tb in range(NTT):
                    xres = finp.tile([128, D], F32, tag="xr")
                    dma_eng().dma_start(xres[:],
                                      x_sh[129 + tb * 128:257 + tb * 128, :])
                    fin = finp.tile([128, D], F32, tag="fin")
                    nc.vector.tensor_add(fin[:], h2f_tm[:, tb, :], xres[:])
                    nc.sync.dma_start(y[tb * 128:(tb + 1) * 128, :], fin[:])
        h2p_ctx.__exit__(None, None, None)


_NC_CACHE = None


def _get_nc():
    global _NC_CACHE
    if _NC_CACHE is None:
        _NC_CACHE = _build_kernel()
    return _NC_CACHE


def _prep_inputs(inputs):
    """Host-side preprocessing: LN-affine folds, diag conv matrices,
    per-core shards."""
    g = {k: np.asarray(v) for k, v in inputs.items()}
    x = g['x'].reshape(N, D).astype(np.float32)
    inv_n = np.float32(g['inv_n'])

    thw = (g['th_ln_g'][:, None] * g['th_w']).astype(np.float32)
    thb = (g['th_b'] + g['th_ln_b'] @ g['th_w']).astype(np.float32)
    qkw = (g['qk_ln_g'][:, None] * g['qk_w']).astype(np.float32)
    qkb = (g['qk_b'] + g['qk_ln_b'] @ g['qk_w']).astype(np.float32)
    outw = (g['out_ln_g'][:, None] * g['out_w']).astype(np.float32)
    outb = (g['out_b'] + g['out_ln_b'] @ g['out_w']).astype(np.float32)
    osg = g['os_gamma'].astype(np.float32).copy()
    osb = g['os_beta'].astype(np.float32).copy()
    osg[0] /= GS
    osb[0] /= GS

    # diag conv matrices (identity tap folded in: +I at k=8) + raw tap
    # vectors for the DVE/GpSimd tap share
    ddm = np.zeros((NCH_ALL, KER, 128, 128), np.float32)
    kerv = np.zeros((NCH_ALL, 128, KER), np.float32)
    kers = [g['th_conv'][:, 0, :], g['qk_conv'][:, 0, :], g['out_conv'][:, 0, :]]
    chunk = 0
    for ker in kers:
        C = ker.shape[0]
        for cb in range(C // 128):
            for k in range(KER):
                v = ker[cb * 128:(cb + 1) * 128, k].copy()
                if k == CP:
                    v = v + 1.0
                np.fill_diagonal(ddm[chunk, k], v)
                kerv[chunk, :, k] = ker[cb * 128:(cb + 1) * 128, k]
            chunk += 1
    assert chunk == NCH_ALL

    xpad = np.zeros((N + 2 * HALO + 1, D), np.float32)
    xpad[HALO + 1:HALO + 1 + N] = x

    import ml_dtypes
    bf = ml_dtypes.bfloat16
    shared = dict(
        thw=thw.astype(bf), thb=thb,
        qkw=qkw.astype(bf), qkb=qkb,
        outw=outw.astype(bf), outb=outb,
        dd=ddm.astype(bf),
        kerv=kerv,
        osg=osg, osb=osb,
        invn=np.full((128, 1), inv_n, np.float32),
    )

    in_maps = []
    for c in range(NC):
        s = c * T
        me = np.ones((128, 256), np.float32)
        if c == 0:
            me[:, :128] = 0.0
        if c == NC - 1:
            me[:, 128:] = 0.0
        selm = np.zeros((128, 16), np.float32)
        for m in range(8):
            if c > 0:
                selm[(c - 1) * 16 + 8 + m, m] = 1.0
            if c < NC - 1:
                selm[(c + 1) * 16 + m, 8 + m] = 1.0
        im = dict(shared)
        im['x_sh'] = np.ascontiguousarray(xpad[s:s + T2 + 1])
        im['medge'] = me
        im['sel'] = selm
        in_maps.append(im)
    return in_maps


def kernel(**inputs):
    nc = _get_nc()
    in_maps = _prep_inputs(inputs)
    res = run_bass_kernel_spmd(nc, in_maps, core_ids=list(range(NC)))
    out = np.concatenate([r['y'] for r in res.results], axis=0)
    return out.reshape(1, N, D).astype(np.float32)


# revision 38
# speedup vs baseline: 1.3610x; 1.3343x over previous
"""FLASH (ShareA, FFConvM) Trainium2 kernel — 8-core SPMD.

Strategy (per the sharding hint): shard the 16384-token sequence across the
8 NeuronCores (2048 tokens each, group-aligned: 8 groups of 256 per core).
Each core computes both FFConvM branches with a 128-token halo so that the
17-tap depthwise convs match the unsharded reference exactly; the global
linear-attention path AllReduces the [128, 1024] lin_kv / lin_ku summaries;
the final FFConvM's depthwise conv exchanges an 8-token boundary halo via a
small AllGather.

All matmuls run in bf16 on the tensor engine (fp32 matmul is 1/4 rate);
LayerNorm statistics, PSUM accumulation, and the residual path stay fp32.
Depthwise convs are evaluated on the PE as 17 PSUM-accumulated matmuls with
per-tap diagonal weight matrices (the identity tap is folded in to add the
conv residual for free). Layout transposes (token-major <-> channel-major)
ride the DMA XBAR transpose engine in 128x128 bf16 blocks.
"""
import sys

if '/opt/trn_rl_repo' not in sys.path:
    sys.path.insert(0, '/opt/trn_rl_repo')

import numpy as np

import concourse.bass as bass
import concourse.tile as tile
from concourse import bacc, mybir
from concourse.bass_utils import run_bass_kernel_spmd

F32 = mybir.dt.float32
BF16 = mybir.dt.bfloat16
AF = mybir.ActivationFunctionType
ALU = mybir.AluOpType

N, D, H2, DV, QK, GS, KER = 16384, 512, 2048, 1024, 128, 256, 17
NC = 8
T = N // NC            # 2048 own tokens
HALO = 128
T2 = T + 2 * HALO      # 2304 pre-activation tokens
CP = (KER - 1) // 2    # 8 (conv halo)
RG = [list(range(NC))]

NCH_H = H2 // 128      # 16 chunks for h
NCH_ALL = NCH_H + 1 + D // 128   # 16 h + 1 qk + 4 out = 21
NTT = T // 128         # 16 token tiles of own range
NLT = T2 // 128        # 18 LN tiles


def _build_kernel():
    nc = bacc.Bacc("TRN2", target_bir_lowering=False, debug=False,
                   num_devices=NC)

    # ---------------- I/O ----------------
    x_sh = nc.dram_tensor("x_sh", [T2 + 1, D], F32, kind="ExternalInput").ap()
    thw = nc.dram_tensor("thw", [D, H2], BF16, kind="ExternalInput").ap()
    thb = nc.dram_tensor("thb", [H2], F32, kind="ExternalInput").ap()
    qkw = nc.dram_tensor("qkw", [D, QK], BF16, kind="ExternalInput").ap()
    qkb = nc.dram_tensor("qkb", [QK], F32, kind="ExternalInput").ap()
    outw = nc.dram_tensor("outw", [DV, D], BF16, kind="ExternalInput").ap()
    outb = nc.dram_tensor("outb", [D], F32, kind="ExternalInput").ap()
    dd = nc.dram_tensor("dd", [NCH_ALL, KER, 128, 128], BF16,
                        kind="ExternalInput").ap()
    osg = nc.dram_tensor("osg", [4, QK], F32, kind="ExternalInput").ap()
    osb = nc.dram_tensor("osb", [4, QK], F32, kind="ExternalInput").ap()
    invn = nc.dram_tensor("invn", [128, 1], F32, kind="ExternalInput").ap()
    medge = nc.dram_tensor("medge", [128, 256], F32, kind="ExternalInput").ap()
    sel = nc.dram_tensor("sel", [128, 16], F32, kind="ExternalInput").ap()
    kerv = nc.dram_tensor("kerv", [NCH_ALL, 128, KER], F32,
                          kind="ExternalInput").ap()
    ident = nc.dram_tensor("ident", [128, 128], BF16,
                           kind="ExternalInput").ap()
    y = nc.dram_tensor("y", [T, D], F32, kind="ExternalOutput").ap()

    with tile.TileContext(nc) as tc:
        _emit(nc, tc, x_sh, thw, thb, qkw, qkb, outw, outb, dd, osg, osb,
              invn, medge, sel, kerv, ident, y)
    nc.compile()
    return nc


DVE_TAPS = ()  # DVE scalar_tensor_tensor only has a 1x uop — PE is 3.4x
GPS_TAPS = ()  # faster per tap, so all taps stay on the tensor engine
PE_TAPS = tuple(k for k in range(KER) if k not in DVE_TAPS + GPS_TAPS)


def _emit(nc, tc, x_sh, thw, thb, qkw, qkb, outw, outb, dd, osg, osb,
          invn, medge, sel, kerv, ident, y):
    from contextlib import ExitStack

    # XBAR transposes stay on the SP (sync) queue — the xbar path is tied
    # to it; plain bulk loads go to the Activation DGE queue so they don't
    # interleave (and mode-switch serialize) with the transposes.
    def dma_eng():
        return nc.scalar

    def dma_t(out, in_):
        nc.sync.dma_start_transpose(out, in_)

    ctx = ExitStack()
    with ctx:
        consts = ctx.enter_context(tc.tile_pool(name="consts", bufs=1))
        mm_ps = ctx.enter_context(tc.tile_pool(name="mm_ps", bufs=2, space="PSUM"))
        sim_ps = ctx.enter_context(tc.tile_pool(name="sim_ps", bufs=1, space="PSUM"))
        att_ps = ctx.enter_context(tc.tile_pool(name="att_ps", bufs=3, space="PSUM"))
        tp_ps = ctx.enter_context(tc.tile_pool(name="tp_ps", bufs=2, space="PSUM"))
        dram = ctx.enter_context(tc.tile_pool(name="dram", bufs=1, space="DRAM"))

        # ------------- constants to SBUF -------------
        qkw_sb = consts.tile([128, D // 128, QK], BF16)
        nc.sync.dma_start(qkw_sb[:], qkw.rearrange("(o p) f -> p o f", p=128))
        outw_sb = consts.tile([128, DV // 128, D], BF16)
        nc.sync.dma_start(outw_sb[:], outw.rearrange("(o p) f -> p o f", p=128))
        thb_sb = consts.tile([128, H2 // 128], F32)
        nc.sync.dma_start(thb_sb[:], thb.rearrange("(o p) -> p o", p=128))
        qkb_sb = consts.tile([128, 1], F32)
        nc.sync.dma_start(qkb_sb[:], qkb.rearrange("(o p) -> p o", p=128))
        outb_sb = consts.tile([128, D // 128], F32)
        nc.sync.dma_start(outb_sb[:], outb.rearrange("(o p) -> p o", p=128))
        osg_sb = consts.tile([128, 4], F32)
        nc.sync.dma_start(osg_sb[:], osg.rearrange("m p -> p m"))
        osb_sb = consts.tile([128, 4], F32)
        nc.sync.dma_start(osb_sb[:], osb.rearrange("m p -> p m"))
        inv_sb = consts.tile([128, 1], F32)
        nc.sync.dma_start(inv_sb[:], invn)
        me_sb = consts.tile([128, 256], F32)
        nc.sync.dma_start(me_sb[:], medge)
        sel_sb = consts.tile([128, 16], F32)
        nc.sync.dma_start(sel_sb[:], sel)
        kerv_sb = consts.tile([128, NCH_ALL, KER], F32)
        nc.sync.dma_start(kerv_sb[:], kerv.rearrange("c p k -> p c k"))
        id_sb = consts.tile([128, 128], BF16)
        nc.sync.dma_start(id_sb[:], ident)
        eps_sb = consts.tile([128, 1], F32)
        nc.vector.memset(eps_sb[:], 1e-5)

        # resident activations (whole-kernel lifetime)
        qs_cm = consts.tile([128, 4, T], BF16)       # [d, m, t']  m: qq,lq,qk_,lk
        lin_kT = consts.tile([128, NTT, QK], BF16)   # [t'%128, tt, d]
        vuT = consts.tile([128, NTT, H2], BF16)      # [t'%128, tt, c] (v|u)
        linkvu_bf = consts.tile([128, 2 * DV], BF16)  # [d, (kv|ku)]

        # ------------- Stage 1: LN + transpose -------------
        with tc.tile_pool(name="thwp", bufs=1) as thwp, \
             tc.tile_pool(name="ln", bufs=3) as lnp, \
             tc.tile_pool(name="xnt", bufs=1) as xntp:
            thw_sb = thwp.tile([128, D // 128, H2], BF16)
            nc.sync.dma_start(thw_sb[:],
                              thw.rearrange("(o p) f -> p o f", p=128))
            xnT = xntp.tile([128, D // 128, T2], BF16)   # [ci%128, ci_chunk, j]
            for tt in range(NLT):
                r0 = tt * 128
                nxt = lnp.tile([128, D], F32, tag="nxt")
                dma_eng().dma_start(nxt[:, 0:D // 2], x_sh[r0:r0 + 128, 0:D // 2])
                dma_eng().dma_start(nxt[:, D // 2:D],
                                  x_sh[r0 + 1:r0 + 129, D // 2:D])
                stats = lnp.tile([128, 6], F32, tag="st")
                nc.vector.bn_stats(stats[:], nxt[:])
                mv = lnp.tile([128, 2], F32, tag="mv")
                nc.vector.bn_aggr(mv[:], stats[:])
                std = lnp.tile([128, 1], F32, tag="sd")
                nc.scalar.activation(std[:], mv[:, 1:2], AF.Sqrt, bias=eps_sb[:])
                nc.vector.reciprocal(std[:], std[:])
                xn = lnp.tile([128, D], BF16, tag="xn")
                nc.vector.tensor_scalar(xn[:], nxt[:], mv[:, 0:1], std[:],
                                        ALU.subtract, ALU.mult)
                # transpose on the (otherwise idle) PE; 4 blocks packed into
                # one bf16 PSUM bank, evacuated with a single strided copy
                tp = tp_ps.tile([128, D // 128, 128], BF16, tag="tp")
                for cb in range(D // 128):
                    nc.tensor.transpose(tp[:, cb, :],
                                        xn[:, cb * 128:(cb + 1) * 128], id_sb[:])
                nc.scalar.activation(xnT[:, :, r0:r0 + 128], tp[:], AF.Copy)

            # ------------- Stage 2+3: linears + convs  -------------
            with tc.tile_pool(name="pre", bufs=3) as prep, \
                 tc.tile_pool(name="ddp", bufs=2) as ddp, \
                 tc.tile_pool(name="hcm", bufs=3) as hcmp:
                # order: qk chunk first (frees the qk/attention path early)
                for ch in range(NCH_H + 1):
                    is_qk = (ch == 0)
                    wch = NCH_H if is_qk else ch - 1   # chunk id in dd
                    pre = prep.tile([128, T2], BF16, tag="pre")
                    for r in range(5):
                        j0 = r * 512
                        w = 512 if r < 4 else T2 - 2048
                        ps = mm_ps.tile([128, 512], F32, tag="mm")
                        for ci in range(D // 128):
                            wsrc = (qkw_sb[:, ci, :] if is_qk else
                                    thw_sb[:, ci,
                                           (wch) * 128:(wch + 1) * 128])
                            nc.tensor.matmul(ps[:, :w], wsrc,
                                             xnT[:, ci, j0:j0 + w],
                                             start=(ci == 0),
                                             stop=(ci == D // 128 - 1))
                        bias = (qkb_sb[:, 0:1] if is_qk else
                                thb_sb[:, wch:wch + 1])
                        nc.scalar.activation(pre[:, j0:j0 + w], ps[:, :w],
                                             AF.Silu, bias=bias)
                    # zero tokens outside the global sequence (edge cores)
                    nc.vector.tensor_mul(pre[:, 0:128], pre[:, 0:128],
                                         me_sb[:, 0:128])
                    nc.vector.tensor_mul(pre[:, T2 - 128:T2],
                                         pre[:, T2 - 128:T2],
                                         me_sb[:, 128:256])

                    dk = ddp.tile([128, KER, 128], BF16, tag="dk")
                    dma_eng().dma_start(dk[:], dd[wch].rearrange("k p f -> p k f"))
                    pe_taps = range(KER) if is_qk else PE_TAPS
                    for q in range(4):
                        ps = mm_ps.tile([128, 512], F32, tag="mm")
                        for i, k in enumerate(pe_taps):
                            nc.tensor.matmul(
                                ps[:], dk[:, k, :],
                                pre[:, 120 + q * 512 + k:632 + q * 512 + k],
                                start=(i == 0), stop=(i == len(pe_taps) - 1))
                        if is_qk:
                            for m in range(4):
                                nc.vector.tensor_scalar(
                                    qs_cm[:, m, q * 512:(q + 1) * 512], ps[:],
                                    osg_sb[:, m:m + 1], osb_sb[:, m:m + 1],
                                    ALU.mult, ALU.add)
                        else:
                            hcm = hcmp.tile([128, 512], BF16, tag="hcm")
                            nc.scalar.activation(hcm[:], ps[:], AF.Copy)
                            # remaining taps on DVE / GpSimd (in-place FMA)
                            for k in DVE_TAPS:
                                nc.vector.scalar_tensor_tensor(
                                    hcm[:],
                                    pre[:, 120 + q * 512 + k:632 + q * 512 + k],
                                    kerv_sb[:, wch, k:k + 1], hcm[:],
                                    ALU.mult, ALU.add)
                            for k in GPS_TAPS:
                                nc.gpsimd.scalar_tensor_tensor(
                                    hcm[:],
                                    pre[:, 120 + q * 512 + k:632 + q * 512 + k],
                                    kerv_sb[:, wch, k:k + 1], hcm[:],
                                    ALU.mult, ALU.add)
                            for b in range(4):
                                tb = q * 4 + b
                                dma_t(
                                    vuT[:, tb, (wch) * 128:(wch + 1) * 128],
                                    hcm[:, b * 128:(b + 1) * 128])
                    if is_qk:
                        for tb in range(NTT):
                            dma_t(
                                lin_kT[:, tb, :],
                                qs_cm[:, 3, tb * 128:(tb + 1) * 128])

        # ------------- Stage 4: lin_kv / lin_ku summaries + AllReduce ----
        # bf16 AllReduce: halves the wire bytes; the summaries only feed the
        # (tiny-magnitude) linear-attention path so bf16 is ample.
        ar_in = dram.tile([128, 2 * DV], BF16)
        ar_out = dram.tile([128, 2 * DV], BF16, addr_space="Shared")
        with tc.tile_pool(name="lkv", bufs=1) as lkvp:
            linkvu = lkvp.tile([128, 2 * DV], BF16)
            for es in range(4):
                ps = mm_ps.tile([128, 512], F32, tag="mm")
                for tb in range(NTT):
                    nc.tensor.matmul(ps[:], lin_kT[:, tb, :],
                                     vuT[:, tb, es * 512:(es + 1) * 512],
                                     start=(tb == 0), stop=(tb == NTT - 1))
                nc.vector.tensor_scalar_mul(linkvu[:, es * 512:(es + 1) * 512],
                                            ps[:], inv_sb[:, 0:1])
            nc.sync.dma_start(ar_in[:], linkvu[:])
        nc.gpsimd.collective_compute("AllReduce", ALU.add, replica_groups=RG,
                                     ins=[ar_in[:]], outs=[ar_out[:]])
        nc.sync.dma_start(linkvu_bf[:], ar_out[:])

        h2p_ctx = tc.tile_pool(name="h2cm", bufs=1)
        h2p = h2p_ctx.__enter__()
        h2_cm = h2p.tile([128, D // 128, T + 2 * CP], BF16)  # [co, ch, z]
        outsbp_ctx = tc.tile_pool(name="outsb", bufs=1)
        outsbp = outsbp_ctx.__enter__()
        out_sb = outsbp.tile([128, NTT, DV], BF16)  # gated out, token-major

        # ------------- Stage 5: quadratic attention + gating -------------
        with tc.tile_pool(name="attn", bufs=3) as attnp, \
             tc.tile_pool(name="gat", bufs=3) as gatp:
            for g in range(T // GS):
                j0 = g * GS
                rr = attnp.tile([128, 2, GS], BF16, tag="rr")
                for jh in range(2):
                    sps = sim_ps.tile([128, GS], F32, tag="sim")
                    nc.tensor.matmul(
                        sps[:], qs_cm[:, 2, j0 + jh * 128:j0 + (jh + 1) * 128],
                        qs_cm[:, 0, j0:j0 + GS], start=True, stop=True)
                    nc.scalar.activation(rr[:, jh, :], sps[:], AF.Relu)
                at = attnp.tile([128, 2, GS], BF16, tag="at")
                nc.vector.tensor_mul(at[:], rr[:], rr[:])
                for ih in range(2):
                    tt = 2 * g + ih
                    psl = []
                    for es in range(4):
                        ps = att_ps.tile([128, 512], F32, tag="att")
                        psl.append(ps)
                        for jh in range(2):
                            nc.tensor.matmul(
                                ps[:], at[:, jh, ih * 128:(ih + 1) * 128],
                                vuT[:, 2 * g + jh, es * 512:(es + 1) * 512],
                                start=(jh == 0), stop=False)
                        nc.tensor.matmul(
                            ps[:], qs_cm[:, 1, j0 + ih * 128:j0 + (ih + 1) * 128],
                            linkvu_bf[:, es * 512:(es + 1) * 512],
                            start=False, stop=True)
                    # gating: out = att_u * v * sigmoid(att_v * u)
                    t1 = gatp.tile([128, DV], BF16, tag="t1")
                    t2 = gatp.tile([128, DV], BF16, tag="t2")
                    for es in range(2):
                        sl = slice(es * 512, (es + 1) * 512)
                        nc.vector.tensor_mul(t1[:, sl], psl[es][:],
                                             vuT[:, tt, DV + es * 512:
                                                 DV + (es + 1) * 512])
                        nc.vector.tensor_mul(t2[:, sl], psl[es + 2][:],
                                             vuT[:, tt, sl])
                    sg = gatp.tile([128, DV], BF16, tag="sg")
                    nc.scalar.activation(sg[:], t1[:], AF.Sigmoid)
                    nc.vector.tensor_mul(out_sb[:, tt, :], t2[:], sg[:])

        # ------------- Stage 6: out-LN + out-linear -------------
        with tc.tile_pool(name="oln", bufs=3) as olnp, \
             tc.tile_pool(name="lnt", bufs=2) as lntp:
            for q in range(4):
                lnoT = lntp.tile([128, DV // 128, 512], BF16, tag="lnoT")
                for it in range(4):
                    tt = q * 4 + it
                    stats = olnp.tile([128, 2, 6], F32, tag="st")
                    nc.vector.bn_stats(stats[:, 0, :], out_sb[:, tt, 0:512])
                    nc.vector.bn_stats(stats[:, 1, :], out_sb[:, tt, 512:DV])
                    mv = olnp.tile([128, 2], F32, tag="mv")
                    nc.vector.bn_aggr(mv[:], stats[:])
                    std = olnp.tile([128, 1], F32, tag="sd")
                    nc.scalar.activation(std[:], mv[:, 1:2], AF.Sqrt,
                                         bias=eps_sb[:])
                    nc.vector.reciprocal(std[:], std[:])
                    lno = olnp.tile([128, DV], BF16, tag="lno")
                    nc.vector.tensor_scalar(lno[:], out_sb[:, tt, :],
                                            mv[:, 0:1], std[:],
                                            ALU.subtract, ALU.mult)
                    for half in range(2):
                        tp = tp_ps.tile([128, 4, 128], BF16, tag="tp")
                        for cb in range(4):
                            nc.tensor.transpose(
                                tp[:, cb, :],
                                lno[:, (half * 4 + cb) * 128:
                                    (half * 4 + cb + 1) * 128], id_sb[:])
                        nc.scalar.activation(
                            lnoT[:, half * 4:half * 4 + 4,
                                 it * 128:(it + 1) * 128], tp[:], AF.Copy)
                for co in range(D // 128):
                    ps = mm_ps.tile([128, 512], F32, tag="mm")
                    for ci in range(DV // 128):
                        nc.tensor.matmul(ps[:],
                                         outw_sb[:, ci, co * 128:(co + 1) * 128],
                                         lnoT[:, ci, :],
                                         start=(ci == 0),
                                         stop=(ci == DV // 128 - 1))
                    nc.scalar.activation(
                        h2_cm[:, co, CP + q * 512:CP + (q + 1) * 512], ps[:],
                        AF.Silu, bias=outb_sb[:, co:co + 1])

        outsbp_ctx.__exit__(None, None, None)

        # ------------- Stage 7: AllGather conv halo -------------
        ag_in = dram.tile([16, D], F32)
        ag_out = dram.tile([NC * 16, D], F32, addr_space="Shared")
        for co in range(D // 128):
            cs = slice(co * 128, (co + 1) * 128)
            nc.gpsimd.dma_start(ag_in[0:8, cs].rearrange("t c -> c t"),
                                h2_cm[:, co, CP:CP + 8])
            nc.gpsimd.dma_start(ag_in[8:16, cs].rearrange("t c -> c t"),
                                h2_cm[:, co, T:T + CP])
        nc.gpsimd.collective_compute("AllGather", ALU.bypass, replica_groups=RG,
                                     ins=[ag_in[:]], outs=[ag_out[:]])
        with tc.tile_pool(name="agp", bufs=1) as agp:
            ag_sb = agp.tile([128, D], F32)
            nc.sync.dma_start(ag_sb[:], ag_out[:])
            for co in range(D // 128):
                hps = att_ps.tile([128, 512], F32, tag="att")
                nc.tensor.matmul(hps[:, 0:16], ag_sb[:, co * 128:(co + 1) * 128],
                                 sel_sb[:], start=True, stop=True)
                nc.vector.tensor_copy(h2_cm[:, co, 0:CP], hps[:, 0:CP])
                nc.vector.tensor_copy(h2_cm[:, co, T + CP:T + 2 * CP],
                                      hps[:, CP:2 * CP])

        # ------------- Stage 8: final conv + residual + store -------------
        # q-outer / co-inner so interior token slabs (q=1,2) fully complete
        # before the AllGather-dependent edge slabs (q=0,3); transposes ride
        # the PE (packed bf16 PSUM) and the residual add reads PSUM directly.
        with tc.tile_pool(name="fddp", bufs=1) as fddp, \
             tc.tile_pool(name="fcm", bufs=6) as fcmp, \
             tc.tile_pool(name="fin", bufs=3) as finp:
            fdd = fddp.tile([128, D // 128, KER, 128], BF16)
            nc.scalar.dma_start(
                fdd[:], dd[NCH_H + 1:NCH_ALL].rearrange("c k p f -> p c k f"))
            for q in (1, 2, 0, 3):
                hcms = []
                for co in range(D // 128):
                    ps = mm_ps.tile([128, 512], F32, tag="mm")
                    for k in range(KER):
                        nc.tensor.matmul(ps[:], fdd[:, co, k, :],
                                         h2_cm[:, co, q * 512 + k:
                                               q * 512 + k + 512],
                                         start=(k == 0), stop=(k == KER - 1))
                    fcm = fcmp.tile([128, 512], BF16, tag="fcm")
                    nc.scalar.activation(fcm[:], ps[:], AF.Copy)
                    hcms.append(fcm)
                for b in range(4):
                    tb = q * 4 + b
                    tp = tp_ps.tile([128, 4, 128], BF16, tag="tp")
                    for co in range(D // 128):
                        nc.tensor.transpose(
                            tp[:, co, :], hcms[co][:, b * 128:(b + 1) * 128],
                            id_sb[:])
                    xres = finp.tile([128, D], F32, tag="xr")
                    dma_eng().dma_start(xres[:],
                                        x_sh[129 + tb * 128:257 + tb * 128, :])
                    fin = finp.tile([128, D], F32, tag="fin")
                    nc.vector.tensor_add(fin[:],
                                         tp[:].rearrange("p a b -> p (a b)"),
                                         xres[:])
                    nc.sync.dma_start(y[tb * 128:(tb + 1) * 128, :], fin[:])
        h2p_ctx.__exit__(None, None, None)


_NC_CACHE = None


def _get_nc():
    global _NC_CACHE
    if _NC_CACHE is None:
        _NC_CACHE = _build_kernel()
    return _NC_CACHE


def _prep_inputs(inputs):
    """Host-side preprocessing: LN-affine folds, diag conv matrices,
    per-core shards."""
    g = {k: np.asarray(v) for k, v in inputs.items()}
    x = g['x'].reshape(N, D).astype(np.float32)
    inv_n = np.float32(g['inv_n'])

    thw = (g['th_ln_g'][:, None] * g['th_w']).astype(np.float32)
    thb = (g['th_b'] + g['th_ln_b'] @ g['th_w']).astype(np.float32)
    qkw = (g['qk_ln_g'][:, None] * g['qk_w']).astype(np.float32)
    qkb = (g['qk_b'] + g['qk_ln_b'] @ g['qk_w']).astype(np.float32)
    outw = (g['out_ln_g'][:, None] * g['out_w']).astype(np.float32)
    outb = (g['out_b'] + g['out_ln_b'] @ g['out_w']).astype(np.float32)
    osg = g['os_gamma'].astype(np.float32).copy()
    osb = g['os_beta'].astype(np.float32).copy()
    osg[0] /= GS
    osb[0] /= GS

    # diag conv matrices (identity tap folded in: +I at k=8) + raw tap
    # vectors for the DVE/GpSimd tap share
    ddm = np.zeros((NCH_ALL, KER, 128, 128), np.float32)
    kerv = np.zeros((NCH_ALL, 128, KER), np.float32)
    kers = [g['th_conv'][:, 0, :], g['qk_conv'][:, 0, :], g['out_conv'][:, 0, :]]
    chunk = 0
    for ker in kers:
        C = ker.shape[0]
        for cb in range(C // 128):
            for k in range(KER):
                v = ker[cb * 128:(cb + 1) * 128, k].copy()
                if k == CP:
                    v = v + 1.0
                np.fill_diagonal(ddm[chunk, k], v)
                kerv[chunk, :, k] = ker[cb * 128:(cb + 1) * 128, k]
            chunk += 1
    assert chunk == NCH_ALL

    xpad = np.zeros((N + 2 * HALO + 1, D), np.float32)
    xpad[HALO + 1:HALO + 1 + N] = x

    import ml_dtypes
    bf = ml_dtypes.bfloat16
    shared = dict(
        thw=thw.astype(bf), thb=thb,
        qkw=qkw.astype(bf), qkb=qkb,
        outw=outw.astype(bf), outb=outb,
        dd=ddm.astype(bf),
        kerv=kerv,
        ident=np.eye(128, dtype=np.float32).astype(bf),
        osg=osg, osb=osb,
        invn=np.full((128, 1), inv_n, np.float32),
    )

    in_maps = []
    for c in range(NC):
        s = c * T
        me = np.ones((128, 256), np.float32)
        if c == 0:
            me[:, :128] = 0.0
        if c == NC - 1:
            me[:, 128:] = 0.0
        selm = np.zeros((128, 16), np.float32)
        for m in range(8):
            if c > 0:
                selm[(c - 1) * 16 + 8 + m, m] = 1.0
            if c < NC - 1:
                selm[(c + 1) * 16 + m, 8 + m] = 1.0
        im = dict(shared)
        im['x_sh'] = np.ascontiguousarray(xpad[s:s + T2 + 1])
        im['medge'] = me
        im['sel'] = selm
        in_maps.append(im)
    return in_maps


def kernel(**inputs):
    nc = _get_nc()
    in_maps = _prep_inputs(inputs)
    res = run_bass_kernel_spmd(nc, in_maps, core_ids=list(range(NC)))
    out = np.concatenate([r['y'] for r in res.results], axis=0)
    return out.reshape(1, N, D).astype(np.float32)


# revision 41
# speedup vs baseline: 1.4584x; 1.0715x over previous
"""FLASH (ShareA, FFConvM) Trainium2 kernel — 8-core SPMD.

Strategy (per the sharding hint): shard the 16384-token sequence across the
8 NeuronCores (2048 tokens each, group-aligned: 8 groups of 256 per core).
Each core computes both FFConvM branches with a 128-token halo so that the
17-tap depthwise convs match the unsharded reference exactly; the global
linear-attention path AllReduces the [128, 1024] lin_kv / lin_ku summaries;
the final FFConvM's depthwise conv exchanges an 8-token boundary halo via a
small AllGather.

All matmuls run in bf16 on the tensor engine (fp32 matmul is 1/4 rate);
LayerNorm statistics, PSUM accumulation, and the residual path stay fp32.
Depthwise convs are evaluated on the PE as 17 PSUM-accumulated matmuls with
per-tap diagonal weight matrices (the identity tap is folded in to add the
conv residual for free). Layout transposes (token-major <-> channel-major)
ride the DMA XBAR transpose engine in 128x128 bf16 blocks.
"""
import sys

if '/opt/trn_rl_repo' not in sys.path:
    sys.path.insert(0, '/opt/trn_rl_repo')

import numpy as np

import concourse.bass as bass
import concourse.tile as tile
from concourse import bacc, mybir
from concourse.bass_utils import run_bass_kernel_spmd

F32 = mybir.dt.float32
BF16 = mybir.dt.bfloat16
AF = mybir.ActivationFunctionType
ALU = mybir.AluOpType

N, D, H2, DV, QK, GS, KER = 16384, 512, 2048, 1024, 128, 256, 17
NC = 8
T = N // NC            # 2048 own tokens
HALO = 128
T2 = T + 2 * HALO      # 2304 pre-activation tokens
CP = (KER - 1) // 2    # 8 (conv halo)
RG = [list(range(NC))]

NCH_H = H2 // 128      # 16 chunks for h
NCH_ALL = NCH_H + 1 + D // 128   # 16 h + 1 qk + 4 out = 21
NTT = T // 128         # 16 token tiles of own range
NLT = T2 // 128        # 18 LN tiles


def _build_kernel():
    nc = bacc.Bacc("TRN2", target_bir_lowering=False, debug=False,
                   num_devices=NC)

    # ---------------- I/O ----------------
    x_sh = nc.dram_tensor("x_sh", [T2 + 1, D], F32, kind="ExternalInput").ap()
    thw = nc.dram_tensor("thw", [D, H2], BF16, kind="ExternalInput").ap()
    thb = nc.dram_tensor("thb", [H2], F32, kind="ExternalInput").ap()
    qkw = nc.dram_tensor("qkw", [D, QK], BF16, kind="ExternalInput").ap()
    qkb = nc.dram_tensor("qkb", [QK], F32, kind="ExternalInput").ap()
    outw = nc.dram_tensor("outw", [DV, D], BF16, kind="ExternalInput").ap()
    outb = nc.dram_tensor("outb", [D], F32, kind="ExternalInput").ap()
    dd = nc.dram_tensor("dd", [NCH_ALL, KER, 128, 128], BF16,
                        kind="ExternalInput").ap()
    osg = nc.dram_tensor("osg", [4, QK], F32, kind="ExternalInput").ap()
    osb = nc.dram_tensor("osb", [4, QK], F32, kind="ExternalInput").ap()
    invn = nc.dram_tensor("invn", [128, 1], F32, kind="ExternalInput").ap()
    medge = nc.dram_tensor("medge", [128, 256], F32, kind="ExternalInput").ap()
    sel = nc.dram_tensor("sel", [128, 16], F32, kind="ExternalInput").ap()
    kerv = nc.dram_tensor("kerv", [NCH_ALL, 128, KER], F32,
                          kind="ExternalInput").ap()
    ident = nc.dram_tensor("ident", [128, 128], BF16,
                           kind="ExternalInput").ap()
    y = nc.dram_tensor("y", [T, D], F32, kind="ExternalOutput").ap()

    with tile.TileContext(nc) as tc:
        _emit(nc, tc, x_sh, thw, thb, qkw, qkb, outw, outb, dd, osg, osb,
              invn, medge, sel, kerv, ident, y)
    nc.compile()
    return nc


DVE_TAPS = ()  # DVE scalar_tensor_tensor only has a 1x uop — PE is 3.4x
GPS_TAPS = ()  # faster per tap, so all taps stay on the tensor engine
PE_TAPS = tuple(k for k in range(KER) if k not in DVE_TAPS + GPS_TAPS)


def _emit(nc, tc, x_sh, thw, thb, qkw, qkb, outw, outb, dd, osg, osb,
          invn, medge, sel, kerv, ident, y):
    from contextlib import ExitStack

    # XBAR transposes stay on the SP (sync) queue — the xbar path is tied
    # to it; plain bulk loads alternate between the two HWDGE queues so
    # they don't mode-switch-serialize with the transposes.
    dma_state = [0]

    def dma_eng():
        dma_state[0] ^= 1
        return nc.scalar if dma_state[0] else nc.sync

    def dma_t(out, in_):
        nc.sync.dma_start_transpose(out, in_)

    ctx = ExitStack()
    with ctx:
        consts = ctx.enter_context(tc.tile_pool(name="consts", bufs=1))
        mm_ps = ctx.enter_context(tc.tile_pool(name="mm_ps", bufs=2, space="PSUM"))
        sim_ps = ctx.enter_context(tc.tile_pool(name="sim_ps", bufs=1, space="PSUM"))
        att_ps = ctx.enter_context(tc.tile_pool(name="att_ps", bufs=3, space="PSUM"))
        tp_ps = ctx.enter_context(tc.tile_pool(name="tp_ps", bufs=2, space="PSUM"))
        dram = ctx.enter_context(tc.tile_pool(name="dram", bufs=1, space="DRAM"))

        # ------------- constants to SBUF -------------
        qkw_sb = consts.tile([128, D // 128, QK], BF16)
        nc.sync.dma_start(qkw_sb[:], qkw.rearrange("(o p) f -> p o f", p=128))
        outw_sb = consts.tile([128, DV // 128, D], BF16)
        nc.sync.dma_start(outw_sb[:], outw.rearrange("(o p) f -> p o f", p=128))
        thb_sb = consts.tile([128, H2 // 128], F32)
        nc.sync.dma_start(thb_sb[:], thb.rearrange("(o p) -> p o", p=128))
        qkb_sb = consts.tile([128, 1], F32)
        nc.sync.dma_start(qkb_sb[:], qkb.rearrange("(o p) -> p o", p=128))
        outb_sb = consts.tile([128, D // 128], F32)
        nc.sync.dma_start(outb_sb[:], outb.rearrange("(o p) -> p o", p=128))
        osg_sb = consts.tile([128, 4], F32)
        nc.sync.dma_start(osg_sb[:], osg.rearrange("m p -> p m"))
        osb_sb = consts.tile([128, 4], F32)
        nc.sync.dma_start(osb_sb[:], osb.rearrange("m p -> p m"))
        inv_sb = consts.tile([128, 1], F32)
        nc.sync.dma_start(inv_sb[:], invn)
        me_sb = consts.tile([128, 256], F32)
        nc.sync.dma_start(me_sb[:], medge)
        sel_sb = consts.tile([128, 16], F32)
        nc.sync.dma_start(sel_sb[:], sel)
        kerv_sb = consts.tile([128, NCH_ALL, KER], F32)
        nc.sync.dma_start(kerv_sb[:], kerv.rearrange("c p k -> p c k"))
        id_sb = consts.tile([128, 128], BF16)
        nc.sync.dma_start(id_sb[:], ident)
        eps_sb = consts.tile([128, 1], F32)
        nc.vector.memset(eps_sb[:], 1e-5)

        # resident activations (whole-kernel lifetime)
        qs_cm = consts.tile([128, 4, T], BF16)       # [d, m, t']  m: qq,lq,qk_,lk
        lin_kT = consts.tile([128, NTT, QK], BF16)   # [t'%128, tt, d]
        vuT = consts.tile([128, NTT, H2], BF16)      # [t'%128, tt, c] (v|u)
        linkvu_bf = consts.tile([128, 2 * DV], BF16)  # [d, (kv|ku)]

        # ------------- Stage 1: LN + transpose -------------
        with tc.tile_pool(name="thwp", bufs=1) as thwp, \
             tc.tile_pool(name="ln", bufs=3) as lnp, \
             tc.tile_pool(name="xnt", bufs=1) as xntp:
            thw_sb = thwp.tile([128, D // 128, H2], BF16)
            nc.sync.dma_start(thw_sb[:],
                              thw.rearrange("(o p) f -> p o f", p=128))
            xnT = xntp.tile([128, D // 128, T2], BF16)   # [ci%128, ci_chunk, j]
            for tt in range(NLT):
                r0 = tt * 128
                nxt = lnp.tile([128, D], F32, tag="nxt")
                dma_eng().dma_start(nxt[:, 0:D // 2], x_sh[r0:r0 + 128, 0:D // 2])
                dma_eng().dma_start(nxt[:, D // 2:D],
                                  x_sh[r0 + 1:r0 + 129, D // 2:D])
                stats = lnp.tile([128, 6], F32, tag="st")
                nc.vector.bn_stats(stats[:], nxt[:])
                mv = lnp.tile([128, 2], F32, tag="mv")
                nc.vector.bn_aggr(mv[:], stats[:])
                std = lnp.tile([128, 1], F32, tag="sd")
                nc.scalar.activation(std[:], mv[:, 1:2], AF.Sqrt, bias=eps_sb[:])
                nc.vector.reciprocal(std[:], std[:])
                xn = lnp.tile([128, D], BF16, tag="xn")
                nc.vector.tensor_scalar(xn[:], nxt[:], mv[:, 0:1], std[:],
                                        ALU.subtract, ALU.mult)
                # transpose on the (otherwise idle) PE; 4 blocks packed into
                # one bf16 PSUM bank, evacuated with a single strided copy
                tp = tp_ps.tile([128, D // 128, 128], BF16, tag="tp")
                for cb in range(D // 128):
                    nc.tensor.transpose(tp[:, cb, :],
                                        xn[:, cb * 128:(cb + 1) * 128], id_sb[:])
                nc.scalar.activation(xnT[:, :, r0:r0 + 128], tp[:], AF.Copy)

            # ------------- Stage 2+3: linears + convs  -------------
            with tc.tile_pool(name="pre", bufs=3) as prep, \
                 tc.tile_pool(name="ddp", bufs=2) as ddp, \
                 tc.tile_pool(name="hcm", bufs=3) as hcmp:
                # order: qk chunk first (frees the qk/attention path early)
                for ch in range(NCH_H + 1):
                    is_qk = (ch == 0)
                    wch = NCH_H if is_qk else ch - 1   # chunk id in dd
                    pre = prep.tile([128, T2], BF16, tag="pre")
                    for r in range(5):
                        j0 = r * 512
                        w = 512 if r < 4 else T2 - 2048
                        ps = mm_ps.tile([128, 512], F32, tag="mm")
                        for ci in range(D // 128):
                            wsrc = (qkw_sb[:, ci, :] if is_qk else
                                    thw_sb[:, ci,
                                           (wch) * 128:(wch + 1) * 128])
                            nc.tensor.matmul(ps[:, :w], wsrc,
                                             xnT[:, ci, j0:j0 + w],
                                             start=(ci == 0),
                                             stop=(ci == D // 128 - 1))
                        bias = (qkb_sb[:, 0:1] if is_qk else
                                thb_sb[:, wch:wch + 1])
                        nc.scalar.activation(pre[:, j0:j0 + w], ps[:, :w],
                                             AF.Silu, bias=bias)
                    # zero tokens outside the global sequence (edge cores)
                    nc.vector.tensor_mul(pre[:, 0:128], pre[:, 0:128],
                                         me_sb[:, 0:128])
                    nc.vector.tensor_mul(pre[:, T2 - 128:T2],
                                         pre[:, T2 - 128:T2],
                                         me_sb[:, 128:256])

                    dk = ddp.tile([128, KER, 128], BF16, tag="dk")
                    dma_eng().dma_start(dk[:], dd[wch].rearrange("k p f -> p k f"))
                    pe_taps = range(KER) if is_qk else PE_TAPS
                    for q in range(4):
                        ps = mm_ps.tile([128, 512], F32, tag="mm")
                        for i, k in enumerate(pe_taps):
                            nc.tensor.matmul(
                                ps[:], dk[:, k, :],
                                pre[:, 120 + q * 512 + k:632 + q * 512 + k],
                                start=(i == 0), stop=(i == len(pe_taps) - 1))
                        if is_qk:
                            for m in range(4):
                                nc.vector.tensor_scalar(
                                    qs_cm[:, m, q * 512:(q + 1) * 512], ps[:],
                                    osg_sb[:, m:m + 1], osb_sb[:, m:m + 1],
                                    ALU.mult, ALU.add)
                        else:
                            hcm = hcmp.tile([128, 512], BF16, tag="hcm")
                            nc.scalar.activation(hcm[:], ps[:], AF.Copy)
                            # transpose: 2 blocks via the XBAR, 2 via the
                            # PE (packed psum + one strided ACT copy) to
                            # split the load between the two resources
                            for b in (0, 2):
                                tb = q * 4 + b
                                dma_t(
                                    vuT[:, tb, (wch) * 128:(wch + 1) * 128],
                                    hcm[:, b * 128:(b + 1) * 128])
                            tp = tp_ps.tile([128, 4, 128], BF16, tag="tp")
                            for i, b in enumerate((1, 3)):
                                nc.tensor.transpose(
                                    tp[:, i, :],
                                    hcm[:, b * 128:(b + 1) * 128], id_sb[:])
                                nc.scalar.activation(
                                    vuT[:, q * 4 + b,
                                        (wch) * 128:(wch + 1) * 128],
                                    tp[:, i, :], AF.Copy)
                    if is_qk:
                        for tb in range(NTT):
                            dma_t(
                                lin_kT[:, tb, :],
                                qs_cm[:, 3, tb * 128:(tb + 1) * 128])

        # ------------- Stage 4: lin_kv / lin_ku summaries + AllReduce ----
        # bf16 AllReduce: halves the wire bytes; the summaries only feed the
        # (tiny-magnitude) linear-attention path so bf16 is ample.
        ar_in = dram.tile([128, 2 * DV], BF16)
        ar_out = dram.tile([128, 2 * DV], BF16, addr_space="Shared")
        with tc.tile_pool(name="lkv", bufs=1) as lkvp:
            linkvu = lkvp.tile([128, 2 * DV], BF16)
            for es in range(4):
                ps = mm_ps.tile([128, 512], F32, tag="mm")
                for tb in range(NTT):
                    nc.tensor.matmul(ps[:], lin_kT[:, tb, :],
                                     vuT[:, tb, es * 512:(es + 1) * 512],
                                     start=(tb == 0), stop=(tb == NTT - 1))
                nc.vector.tensor_scalar_mul(linkvu[:, es * 512:(es + 1) * 512],
                                            ps[:], inv_sb[:, 0:1])
            nc.sync.dma_start(ar_in[:], linkvu[:])
        nc.gpsimd.collective_compute("AllReduce", ALU.add, replica_groups=RG,
                                     ins=[ar_in[:]], outs=[ar_out[:]])
        nc.sync.dma_start(linkvu_bf[:], ar_out[:])

        h2p_ctx = tc.tile_pool(name="h2cm", bufs=1)
        h2p = h2p_ctx.__enter__()
        h2_cm = h2p.tile([128, D // 128, T + 2 * CP], BF16)  # [co, ch, z]
        outsbp_ctx = tc.tile_pool(name="outsb", bufs=1)
        outsbp = outsbp_ctx.__enter__()
        out_sb = outsbp.tile([128, NTT, DV], BF16)  # gated out, token-major

        # ------------- Stage 5: quadratic attention + gating -------------
        with tc.tile_pool(name="attn", bufs=3) as attnp, \
             tc.tile_pool(name="gat", bufs=3) as gatp:
            for g in range(T // GS):
                j0 = g * GS
                rr = attnp.tile([128, 2, GS], BF16, tag="rr")
                for jh in range(2):
                    sps = sim_ps.tile([128, GS], F32, tag="sim")
                    nc.tensor.matmul(
                        sps[:], qs_cm[:, 2, j0 + jh * 128:j0 + (jh + 1) * 128],
                        qs_cm[:, 0, j0:j0 + GS], start=True, stop=True)
                    nc.scalar.activation(rr[:, jh, :], sps[:], AF.Relu)
                at = attnp.tile([128, 2, GS], BF16, tag="at")
                nc.vector.tensor_mul(at[:], rr[:], rr[:])
                for ih in range(2):
                    tt = 2 * g + ih
                    psl = []
                    for es in range(4):
                        ps = att_ps.tile([128, 512], F32, tag="att")
                        psl.append(ps)
                        for jh in range(2):
                            nc.tensor.matmul(
                                ps[:], at[:, jh, ih * 128:(ih + 1) * 128],
                                vuT[:, 2 * g + jh, es * 512:(es + 1) * 512],
                                start=(jh == 0), stop=False)
                        nc.tensor.matmul(
                            ps[:], qs_cm[:, 1, j0 + ih * 128:j0 + (ih + 1) * 128],
                            linkvu_bf[:, es * 512:(es + 1) * 512],
                            start=False, stop=True)
                    # gating: out = att_u * v * sigmoid(att_v * u)
                    t1 = gatp.tile([128, DV], BF16, tag="t1")
                    t2 = gatp.tile([128, DV], BF16, tag="t2")
                    for es in range(2):
                        sl = slice(es * 512, (es + 1) * 512)
                        nc.vector.tensor_mul(t1[:, sl], psl[es][:],
                                             vuT[:, tt, DV + es * 512:
                                                 DV + (es + 1) * 512])
                        nc.vector.tensor_mul(t2[:, sl], psl[es + 2][:],
                                             vuT[:, tt, sl])
                    sg = gatp.tile([128, DV], BF16, tag="sg")
                    nc.scalar.activation(sg[:], t1[:], AF.Sigmoid)
                    nc.vector.tensor_mul(out_sb[:, tt, :], t2[:], sg[:])

        # ------------- Stage 6: out-LN + out-linear -------------
        with tc.tile_pool(name="oln", bufs=3) as olnp, \
             tc.tile_pool(name="lnt", bufs=2) as lntp:
            for q in range(4):
                lnoT = lntp.tile([128, DV // 128, 512], BF16, tag="lnoT")
                for it in range(4):
                    tt = q * 4 + it
                    stats = olnp.tile([128, 2, 6], F32, tag="st")
                    nc.vector.bn_stats(stats[:, 0, :], out_sb[:, tt, 0:512])
                    nc.vector.bn_stats(stats[:, 1, :], out_sb[:, tt, 512:DV])
                    mv = olnp.tile([128, 2], F32, tag="mv")
                    nc.vector.bn_aggr(mv[:], stats[:])
                    std = olnp.tile([128, 1], F32, tag="sd")
                    nc.scalar.activation(std[:], mv[:, 1:2], AF.Sqrt,
                                         bias=eps_sb[:])
                    nc.vector.reciprocal(std[:], std[:])
                    lno = olnp.tile([128, DV], BF16, tag="lno")
                    nc.vector.tensor_scalar(lno[:], out_sb[:, tt, :],
                                            mv[:, 0:1], std[:],
                                            ALU.subtract, ALU.mult)
                    for half in range(2):
                        tp = tp_ps.tile([128, 4, 128], BF16, tag="tp")
                        for cb in range(4):
                            nc.tensor.transpose(
                                tp[:, cb, :],
                                lno[:, (half * 4 + cb) * 128:
                                    (half * 4 + cb + 1) * 128], id_sb[:])
                        nc.scalar.activation(
                            lnoT[:, half * 4:half * 4 + 4,
                                 it * 128:(it + 1) * 128], tp[:], AF.Copy)
                for co in range(D // 128):
                    ps = mm_ps.tile([128, 512], F32, tag="mm")
                    for ci in range(DV // 128):
                        nc.tensor.matmul(ps[:],
                                         outw_sb[:, ci, co * 128:(co + 1) * 128],
                                         lnoT[:, ci, :],
                                         start=(ci == 0),
                                         stop=(ci == DV // 128 - 1))
                    nc.scalar.activation(
                        h2_cm[:, co, CP + q * 512:CP + (q + 1) * 512], ps[:],
                        AF.Silu, bias=outb_sb[:, co:co + 1])

        outsbp_ctx.__exit__(None, None, None)

        # ------------- Stage 7: AllGather conv halo -------------
        ag_in = dram.tile([16, D], F32)
        ag_out = dram.tile([NC * 16, D], F32, addr_space="Shared")
        for co in range(D // 128):
            cs = slice(co * 128, (co + 1) * 128)
            nc.gpsimd.dma_start(ag_in[0:8, cs].rearrange("t c -> c t"),
                                h2_cm[:, co, CP:CP + 8])
            nc.gpsimd.dma_start(ag_in[8:16, cs].rearrange("t c -> c t"),
                                h2_cm[:, co, T:T + CP])
        nc.gpsimd.collective_compute("AllGather", ALU.bypass, replica_groups=RG,
                                     ins=[ag_in[:]], outs=[ag_out[:]])
        with tc.tile_pool(name="agp", bufs=1) as agp:
            ag_sb = agp.tile([128, D], F32)
            nc.sync.dma_start(ag_sb[:], ag_out[:])
            for co in range(D // 128):
                hps = att_ps.tile([128, 512], F32, tag="att")
                nc.tensor.matmul(hps[:, 0:16], ag_sb[:, co * 128:(co + 1) * 128],
                                 sel_sb[:], start=True, stop=True)
                nc.vector.tensor_copy(h2_cm[:, co, 0:CP], hps[:, 0:CP])
                nc.vector.tensor_copy(h2_cm[:, co, T + CP:T + 2 * CP],
                                      hps[:, CP:2 * CP])

        # ------------- Stage 8: final conv + residual + store -------------
        # q-outer / co-inner so interior token slabs (q=1,2) fully complete
        # before the AllGather-dependent edge slabs (q=0,3); transposes ride
        # the PE (packed bf16 PSUM) and the residual add reads PSUM directly.
        with tc.tile_pool(name="fddp", bufs=1) as fddp, \
             tc.tile_pool(name="fcm", bufs=6) as fcmp, \
             tc.tile_pool(name="fin", bufs=3) as finp:
            fdd = fddp.tile([128, D // 128, KER, 128], BF16)
            nc.scalar.dma_start(
                fdd[:], dd[NCH_H + 1:NCH_ALL].rearrange("c k p f -> p c k f"))
            for q in (1, 2, 0, 3):
                hcms = []
                for co in range(D // 128):
                    ps = mm_ps.tile([128, 512], F32, tag="mm")
                    for k in range(KER):
                        nc.tensor.matmul(ps[:], fdd[:, co, k, :],
                                         h2_cm[:, co, q * 512 + k:
                                               q * 512 + k + 512],
                                         start=(k == 0), stop=(k == KER - 1))
                    fcm = fcmp.tile([128, 512], BF16, tag="fcm")
                    nc.scalar.activation(fcm[:], ps[:], AF.Copy)
                    hcms.append(fcm)
                for b in range(4):
                    tb = q * 4 + b
                    tp = tp_ps.tile([128, 4, 128], BF16, tag="tp")
                    for co in range(D // 128):
                        nc.tensor.transpose(
                            tp[:, co, :], hcms[co][:, b * 128:(b + 1) * 128],
                            id_sb[:])
                    xres = finp.tile([128, D], F32, tag="xr")
                    dma_eng().dma_start(xres[:],
                                        x_sh[129 + tb * 128:257 + tb * 128, :])
                    fin = finp.tile([128, D], F32, tag="fin")
                    nc.vector.tensor_add(fin[:],
                                         tp[:].rearrange("p a b -> p (a b)"),
                                         xres[:])
                    nc.sync.dma_start(y[tb * 128:(tb + 1) * 128, :], fin[:])
        h2p_ctx.__exit__(None, None, None)


_NC_CACHE = None


def _get_nc():
    global _NC_CACHE
    if _NC_CACHE is None:
        _NC_CACHE = _build_kernel()
    return _NC_CACHE


def _prep_inputs(inputs):
    """Host-side preprocessing: LN-affine folds, diag conv matrices,
    per-core shards."""
    g = {k: np.asarray(v) for k, v in inputs.items()}
    x = g['x'].reshape(N, D).astype(np.float32)
    inv_n = np.float32(g['inv_n'])

    thw = (g['th_ln_g'][:, None] * g['th_w']).astype(np.float32)
    thb = (g['th_b'] + g['th_ln_b'] @ g['th_w']).astype(np.float32)
    qkw = (g['qk_ln_g'][:, None] * g['qk_w']).astype(np.float32)
    qkb = (g['qk_b'] + g['qk_ln_b'] @ g['qk_w']).astype(np.float32)
    outw = (g['out_ln_g'][:, None] * g['out_w']).astype(np.float32)
    outb = (g['out_b'] + g['out_ln_b'] @ g['out_w']).astype(np.float32)
    osg = g['os_gamma'].astype(np.float32).copy()
    osb = g['os_beta'].astype(np.float32).copy()
    osg[0] /= GS
    osb[0] /= GS

    # diag conv matrices (identity tap folded in: +I at k=8) + raw tap
    # vectors for the DVE/GpSimd tap share
    ddm = np.zeros((NCH_ALL, KER, 128, 128), np.float32)
    kerv = np.zeros((NCH_ALL, 128, KER), np.float32)
    kers = [g['th_conv'][:, 0, :], g['qk_conv'][:, 0, :], g['out_conv'][:, 0, :]]
    chunk = 0
    for ker in kers:
        C = ker.shape[0]
        for cb in range(C // 128):
            for k in range(KER):
                v = ker[cb * 128:(cb + 1) * 128, k].copy()
                if k == CP:
                    v = v + 1.0
                np.fill_diagonal(ddm[chunk, k], v)
                kerv[chunk, :, k] = ker[cb * 128:(cb + 1) * 128, k]
            chunk += 1
    assert chunk == NCH_ALL

    xpad = np.zeros((N + 2 * HALO + 1, D), np.float32)
    xpad[HALO + 1:HALO + 1 + N] = x

    import ml_dtypes
    bf = ml_dtypes.bfloat16
    shared = dict(
        thw=thw.astype(bf), thb=thb,
        qkw=qkw.astype(bf), qkb=qkb,
        outw=outw.astype(bf), outb=outb,
        dd=ddm.astype(bf),
        kerv=kerv,
        ident=np.eye(128, dtype=np.float32).astype(bf),
        osg=osg, osb=osb,
        invn=np.full((128, 1), inv_n, np.float32),
    )

    in_maps = []
    for c in range(NC):
        s = c * T
        me = np.ones((128, 256), np.float32)
        if c == 0:
            me[:, :128] = 0.0
        if c == NC - 1:
            me[:, 128:] = 0.0
        selm = np.zeros((128, 16), np.float32)
        for m in range(8):
            if c > 0:
                selm[(c - 1) * 16 + 8 + m, m] = 1.0
            if c < NC - 1:
                selm[(c + 1) * 16 + m, 8 + m] = 1.0
        im = dict(shared)
        im['x_sh'] = np.ascontiguousarray(xpad[s:s + T2 + 1])
        im['medge'] = me
        im['sel'] = selm
        in_maps.append(im)
    return in_maps


def kernel(**inputs):
    nc = _get_nc()
    in_maps = _prep_inputs(inputs)
    res = run_bass_kernel_spmd(nc, in_maps, core_ids=list(range(NC)))
    out = np.concatenate([r['y'] for r in res.results], axis=0)
    return out.reshape(1, N, D).astype(np.float32)


# revision 46
# speedup vs baseline: 1.4677x; 1.0064x over previous
"""FLASH (ShareA, FFConvM) Trainium2 kernel — 8-core SPMD.

Strategy (per the sharding hint): shard the 16384-token sequence across the
8 NeuronCores (2048 tokens each, group-aligned: 8 groups of 256 per core).
Each core computes both FFConvM branches with a 128-token halo so that the
17-tap depthwise convs match the unsharded reference exactly; the global
linear-attention path AllReduces the [128, 1024] lin_kv / lin_ku summaries;
the final FFConvM's depthwise conv exchanges an 8-token boundary halo via a
small AllGather.

All matmuls run in bf16 on the tensor engine (fp32 matmul is 1/4 rate);
LayerNorm statistics, PSUM accumulation, and the residual path stay fp32.
Depthwise convs are evaluated on the PE as 17 PSUM-accumulated matmuls with
per-tap diagonal weight matrices plus one exact-identity matmul for the
conv residual (keeping 1+k out of bf16). Layout transposes (token-major <->
channel-major) are split between the DMA XBAR transpose engine (sync queue)
and PE transposes evacuated through packed bf16 PSUM banks, so neither
resource serializes the pipeline.
"""
import sys

if '/opt/trn_rl_repo' not in sys.path:
    sys.path.insert(0, '/opt/trn_rl_repo')

import numpy as np

import concourse.bass as bass
import concourse.tile as tile
from concourse import bacc, mybir
from concourse.bass_utils import run_bass_kernel_spmd

F32 = mybir.dt.float32
BF16 = mybir.dt.bfloat16
AF = mybir.ActivationFunctionType
ALU = mybir.AluOpType

N, D, H2, DV, QK, GS, KER = 16384, 512, 2048, 1024, 128, 256, 17
NC = 8
T = N // NC            # 2048 own tokens
HALO = 128
T2 = T + 2 * HALO      # 2304 pre-activation tokens
CP = (KER - 1) // 2    # 8 (conv halo)
RG = [list(range(NC))]

NCH_H = H2 // 128      # 16 chunks for h
NCH_ALL = NCH_H + 1 + D // 128   # 16 h + 1 qk + 4 out = 21
NTT = T // 128         # 16 token tiles of own range
NLT = T2 // 128        # 18 LN tiles


def _build_kernel():
    nc = bacc.Bacc("TRN2", target_bir_lowering=False, debug=False,
                   num_devices=NC)

    # ---------------- I/O ----------------
    x_sh = nc.dram_tensor("x_sh", [T2 + 1, D], F32, kind="ExternalInput").ap()
    thw = nc.dram_tensor("thw", [D, H2], BF16, kind="ExternalInput").ap()
    thb = nc.dram_tensor("thb", [H2], F32, kind="ExternalInput").ap()
    qkw = nc.dram_tensor("qkw", [D, QK], BF16, kind="ExternalInput").ap()
    qkb = nc.dram_tensor("qkb", [QK], F32, kind="ExternalInput").ap()
    outw = nc.dram_tensor("outw", [DV, D], BF16, kind="ExternalInput").ap()
    outb = nc.dram_tensor("outb", [D], F32, kind="ExternalInput").ap()
    dd = nc.dram_tensor("dd", [NCH_ALL, KER, 128, 128], BF16,
                        kind="ExternalInput").ap()
    osg = nc.dram_tensor("osg", [4, QK], F32, kind="ExternalInput").ap()
    osb = nc.dram_tensor("osb", [4, QK], F32, kind="ExternalInput").ap()
    invn = nc.dram_tensor("invn", [128, 1], F32, kind="ExternalInput").ap()
    medge = nc.dram_tensor("medge", [128, 256], F32, kind="ExternalInput").ap()
    sel = nc.dram_tensor("sel", [128, 16], F32, kind="ExternalInput").ap()
    kerv = nc.dram_tensor("kerv", [NCH_ALL, 128, KER], F32,
                          kind="ExternalInput").ap()
    ident = nc.dram_tensor("ident", [128, 128], BF16,
                           kind="ExternalInput").ap()
    y = nc.dram_tensor("y", [T, D], F32, kind="ExternalOutput").ap()

    with tile.TileContext(nc) as tc:
        _emit(nc, tc, x_sh, thw, thb, qkw, qkb, outw, outb, dd, osg, osb,
              invn, medge, sel, kerv, ident, y)
    nc.compile()
    return nc


DVE_TAPS = ()  # DVE scalar_tensor_tensor only has a 1x uop — PE is 3.4x
GPS_TAPS = ()  # faster per tap, so all taps stay on the tensor engine
PE_TAPS = tuple(k for k in range(KER) if k not in DVE_TAPS + GPS_TAPS)


def _emit(nc, tc, x_sh, thw, thb, qkw, qkb, outw, outb, dd, osg, osb,
          invn, medge, sel, kerv, ident, y):
    from contextlib import ExitStack

    # XBAR transposes stay on the SP (sync) queue — the xbar path is tied
    # to it; plain bulk loads alternate between the two HWDGE queues so
    # they don't mode-switch-serialize with the transposes.
    dma_state = [0]

    def dma_eng():
        dma_state[0] ^= 1
        return nc.scalar if dma_state[0] else nc.sync

    def dma_t(out, in_):
        nc.sync.dma_start_transpose(out, in_)

    ctx = ExitStack()
    with ctx:
        consts = ctx.enter_context(tc.tile_pool(name="consts", bufs=1))
        mm_ps = ctx.enter_context(tc.tile_pool(name="mm_ps", bufs=2, space="PSUM"))
        sim_ps = ctx.enter_context(tc.tile_pool(name="sim_ps", bufs=1, space="PSUM"))
        att_ps = ctx.enter_context(tc.tile_pool(name="att_ps", bufs=3, space="PSUM"))
        tp_ps = ctx.enter_context(tc.tile_pool(name="tp_ps", bufs=2, space="PSUM"))
        dram = ctx.enter_context(tc.tile_pool(name="dram", bufs=1, space="DRAM"))

        # ------------- constants to SBUF -------------
        qkw_sb = consts.tile([128, D // 128, QK], BF16)
        nc.sync.dma_start(qkw_sb[:], qkw.rearrange("(o p) f -> p o f", p=128))
        outw_sb = consts.tile([128, DV // 128, D], BF16)
        nc.sync.dma_start(outw_sb[:], outw.rearrange("(o p) f -> p o f", p=128))
        thb_sb = consts.tile([128, H2 // 128], F32)
        nc.sync.dma_start(thb_sb[:], thb.rearrange("(o p) -> p o", p=128))
        qkb_sb = consts.tile([128, 1], F32)
        nc.sync.dma_start(qkb_sb[:], qkb.rearrange("(o p) -> p o", p=128))
        outb_sb = consts.tile([128, D // 128], F32)
        nc.sync.dma_start(outb_sb[:], outb.rearrange("(o p) -> p o", p=128))
        osg_sb = consts.tile([128, 4], F32)
        nc.sync.dma_start(osg_sb[:], osg.rearrange("m p -> p m"))
        osb_sb = consts.tile([128, 4], F32)
        nc.sync.dma_start(osb_sb[:], osb.rearrange("m p -> p m"))
        inv_sb = consts.tile([128, 1], F32)
        nc.sync.dma_start(inv_sb[:], invn)
        me_sb = consts.tile([128, 256], F32)
        nc.sync.dma_start(me_sb[:], medge)
        sel_sb = consts.tile([128, 16], F32)
        nc.sync.dma_start(sel_sb[:], sel)
        kerv_sb = consts.tile([128, NCH_ALL, KER], F32)
        nc.sync.dma_start(kerv_sb[:], kerv.rearrange("c p k -> p c k"))
        id_sb = consts.tile([128, 128], BF16)
        nc.sync.dma_start(id_sb[:], ident)
        id32_sb = consts.tile([128, 128], F32)
        nc.vector.tensor_copy(id32_sb[:], id_sb[:])
        eps_sb = consts.tile([128, 1], F32)
        nc.vector.memset(eps_sb[:], 1e-5)

        # resident activations (whole-kernel lifetime)
        qs_cm = consts.tile([128, 4, T], BF16)       # [d, m, t']  m: qq,lq,qk_,lk
        lin_kT = consts.tile([128, NTT, QK], BF16)   # [t'%128, tt, d]
        vuT = consts.tile([128, NTT, H2], BF16)      # [t'%128, tt, c] (v|u)
        linkvu_bf = consts.tile([128, 2 * DV], BF16)  # [d, (kv|ku)]

        # ------------- Stage 1: LN + transpose -------------
        with tc.tile_pool(name="thwp", bufs=1) as thwp, \
             tc.tile_pool(name="ln", bufs=3) as lnp, \
             tc.tile_pool(name="xnt", bufs=1) as xntp:
            thw_sb = thwp.tile([128, D // 128, H2], BF16)
            nc.sync.dma_start(thw_sb[:],
                              thw.rearrange("(o p) f -> p o f", p=128))
            xnT = xntp.tile([128, D // 128, T2], BF16)   # [ci%128, ci_chunk, j]
            for tt in range(NLT):
                r0 = tt * 128
                nxt = lnp.tile([128, D], F32, tag="nxt")
                dma_eng().dma_start(nxt[:, 0:D // 2], x_sh[r0:r0 + 128, 0:D // 2])
                dma_eng().dma_start(nxt[:, D // 2:D],
                                  x_sh[r0 + 1:r0 + 129, D // 2:D])
                stats = lnp.tile([128, 6], F32, tag="st")
                nc.vector.bn_stats(stats[:], nxt[:])
                mv = lnp.tile([128, 2], F32, tag="mv")
                nc.vector.bn_aggr(mv[:], stats[:])
                std = lnp.tile([128, 1], F32, tag="sd")
                nc.scalar.activation(std[:], mv[:, 1:2], AF.Sqrt, bias=eps_sb[:])
                nc.vector.reciprocal(std[:], std[:])
                xn = lnp.tile([128, D], BF16, tag="xn")
                nc.vector.tensor_scalar(xn[:], nxt[:], mv[:, 0:1], std[:],
                                        ALU.subtract, ALU.mult)
                # transpose on the (otherwise idle) PE; 4 blocks packed into
                # one bf16 PSUM bank, evacuated with a single strided copy
                tp = tp_ps.tile([128, D // 128, 128], BF16, tag="tp")
                for cb in range(D // 128):
                    nc.tensor.transpose(tp[:, cb, :],
                                        xn[:, cb * 128:(cb + 1) * 128], id_sb[:])
                nc.scalar.activation(xnT[:, :, r0:r0 + 128], tp[:], AF.Copy)

            # ------------- Stage 2+3: linears + convs  -------------
            with tc.tile_pool(name="pre", bufs=3) as prep, \
                 tc.tile_pool(name="ddp", bufs=2) as ddp, \
                 tc.tile_pool(name="hcm", bufs=3) as hcmp:
                # order: qk chunk first (frees the qk/attention path early)
                for ch in range(NCH_H + 1):
                    is_qk = (ch == 0)
                    wch = NCH_H if is_qk else ch - 1   # chunk id in dd
                    pre = prep.tile([128, T2], BF16, tag="pre")
                    for r in range(5):
                        j0 = r * 512
                        w = 512 if r < 4 else T2 - 2048
                        ps = mm_ps.tile([128, 512], F32, tag="mm")
                        for ci in range(D // 128):
                            wsrc = (qkw_sb[:, ci, :] if is_qk else
                                    thw_sb[:, ci,
                                           (wch) * 128:(wch + 1) * 128])
                            nc.tensor.matmul(ps[:, :w], wsrc,
                                             xnT[:, ci, j0:j0 + w],
                                             start=(ci == 0),
                                             stop=(ci == D // 128 - 1))
                        bias = (qkb_sb[:, 0:1] if is_qk else
                                thb_sb[:, wch:wch + 1])
                        nc.scalar.activation(pre[:, j0:j0 + w], ps[:, :w],
                                             AF.Silu, bias=bias)
                    # zero tokens outside the global sequence (edge cores)
                    nc.vector.tensor_mul(pre[:, 0:128], pre[:, 0:128],
                                         me_sb[:, 0:128])
                    nc.vector.tensor_mul(pre[:, T2 - 128:T2],
                                         pre[:, T2 - 128:T2],
                                         me_sb[:, 128:256])

                    dk = ddp.tile([128, KER, 128], BF16, tag="dk")
                    dma_eng().dma_start(dk[:], dd[wch].rearrange("k p f -> p k f"))
                    pe_taps = range(KER) if is_qk else PE_TAPS
                    for q in range(4):
                        ps = mm_ps.tile([128, 512], F32, tag="mm")
                        for k in pe_taps:
                            nc.tensor.matmul(
                                ps[:], dk[:, k, :],
                                pre[:, 120 + q * 512 + k:632 + q * 512 + k],
                                start=(k == pe_taps[0] if not is_qk else k == 0),
                                stop=False)
                        # conv residual via an exact bf16 identity matmul
                        nc.tensor.matmul(
                            ps[:], id_sb[:],
                            pre[:, 128 + q * 512:640 + q * 512],
                            start=False, stop=True)
                        if is_qk:
                            for m in range(4):
                                nc.vector.tensor_scalar(
                                    qs_cm[:, m, q * 512:(q + 1) * 512], ps[:],
                                    osg_sb[:, m:m + 1], osb_sb[:, m:m + 1],
                                    ALU.mult, ALU.add)
                        else:
                            hcm = hcmp.tile([128, 512], BF16, tag="hcm")
                            nc.scalar.activation(hcm[:], ps[:], AF.Copy)
                            # transpose: 2 blocks via the XBAR, 2 via the
                            # PE (packed psum + one strided ACT copy) to
                            # split the load between the two resources
                            for b in (0, 2):
                                tb = q * 4 + b
                                dma_t(
                                    vuT[:, tb, (wch) * 128:(wch + 1) * 128],
                                    hcm[:, b * 128:(b + 1) * 128])
                            tp = tp_ps.tile([128, 4, 128], BF16, tag="tp")
                            for i, b in enumerate((1, 3)):
                                nc.tensor.transpose(
                                    tp[:, i, :],
                                    hcm[:, b * 128:(b + 1) * 128], id_sb[:])
                                nc.scalar.activation(
                                    vuT[:, q * 4 + b,
                                        (wch) * 128:(wch + 1) * 128],
                                    tp[:, i, :], AF.Copy)
                    if is_qk:
                        for tb in range(NTT):
                            dma_t(
                                lin_kT[:, tb, :],
                                qs_cm[:, 3, tb * 128:(tb + 1) * 128])

        # ------------- Stage 4: lin_kv / lin_ku summaries + AllReduce ----
        # bf16 AllReduce: halves the wire bytes; the summaries only feed the
        # (tiny-magnitude) linear-attention path so bf16 is ample.
        ar_in = dram.tile([128, 2 * DV], BF16)
        ar_out = dram.tile([128, 2 * DV], BF16, addr_space="Shared")
        with tc.tile_pool(name="lkv", bufs=1) as lkvp:
            linkvu = lkvp.tile([128, 2 * DV], BF16)
            for es in range(4):
                ps = mm_ps.tile([128, 512], F32, tag="mm")
                for tb in range(NTT):
                    nc.tensor.matmul(ps[:], lin_kT[:, tb, :],
                                     vuT[:, tb, es * 512:(es + 1) * 512],
                                     start=(tb == 0), stop=(tb == NTT - 1))
                nc.vector.tensor_scalar_mul(linkvu[:, es * 512:(es + 1) * 512],
                                            ps[:], inv_sb[:, 0:1])
            nc.sync.dma_start(ar_in[:], linkvu[:])
        nc.gpsimd.collective_compute("AllReduce", ALU.add, replica_groups=RG,
                                     ins=[ar_in[:]], outs=[ar_out[:]])
        nc.sync.dma_start(linkvu_bf[:], ar_out[:])

        h2p_ctx = tc.tile_pool(name="h2cm", bufs=1)
        h2p = h2p_ctx.__enter__()
        h2_cm = h2p.tile([128, D // 128, T + 2 * CP], BF16)  # [co, ch, z]
        outsbp_ctx = tc.tile_pool(name="outsb", bufs=1)
        outsbp = outsbp_ctx.__enter__()
        out_sb = outsbp.tile([128, NTT, DV], BF16)  # gated out, token-major

        # ------------- Stage 5: quadratic attention + gating -------------
        with tc.tile_pool(name="attn", bufs=3) as attnp, \
             tc.tile_pool(name="gat", bufs=3) as gatp:
            for g in range(T // GS):
                j0 = g * GS
                rr = attnp.tile([128, 2, GS], BF16, tag="rr")
                for jh in range(2):
                    sps = sim_ps.tile([128, GS], F32, tag="sim")
                    nc.tensor.matmul(
                        sps[:], qs_cm[:, 2, j0 + jh * 128:j0 + (jh + 1) * 128],
                        qs_cm[:, 0, j0:j0 + GS], start=True, stop=True)
                    nc.scalar.activation(rr[:, jh, :], sps[:], AF.Relu)
                at = attnp.tile([128, 2, GS], BF16, tag="at")
                nc.vector.tensor_mul(at[:], rr[:], rr[:])
                for ih in range(2):
                    tt = 2 * g + ih
                    psl = []
                    for es in range(4):
                        ps = att_ps.tile([128, 512], F32, tag="att")
                        psl.append(ps)
                        for jh in range(2):
                            nc.tensor.matmul(
                                ps[:], at[:, jh, ih * 128:(ih + 1) * 128],
                                vuT[:, 2 * g + jh, es * 512:(es + 1) * 512],
                                start=(jh == 0), stop=False)
                        nc.tensor.matmul(
                            ps[:], qs_cm[:, 1, j0 + ih * 128:j0 + (ih + 1) * 128],
                            linkvu_bf[:, es * 512:(es + 1) * 512],
                            start=False, stop=True)
                    # gating: out = att_u * v * sigmoid(att_v * u)
                    t1 = gatp.tile([128, DV], BF16, tag="t1")
                    t2 = gatp.tile([128, DV], BF16, tag="t2")
                    for es in range(2):
                        sl = slice(es * 512, (es + 1) * 512)
                        nc.vector.tensor_mul(t1[:, sl], psl[es][:],
                                             vuT[:, tt, DV + es * 512:
                                                 DV + (es + 1) * 512])
                        nc.vector.tensor_mul(t2[:, sl], psl[es + 2][:],
                                             vuT[:, tt, sl])
                    sg = gatp.tile([128, DV], BF16, tag="sg")
                    nc.scalar.activation(sg[:], t1[:], AF.Sigmoid)
                    nc.vector.tensor_mul(out_sb[:, tt, :], t2[:], sg[:])

        # ------------- Stage 6: out-LN + out-linear -------------
        with tc.tile_pool(name="oln", bufs=3) as olnp, \
             tc.tile_pool(name="lnt", bufs=2) as lntp:
            for q in range(4):
                lnoT = lntp.tile([128, DV // 128, 512], BF16, tag="lnoT")
                for it in range(4):
                    tt = q * 4 + it
                    stats = olnp.tile([128, 2, 6], F32, tag="st")
                    nc.vector.bn_stats(stats[:, 0, :], out_sb[:, tt, 0:512])
                    nc.vector.bn_stats(stats[:, 1, :], out_sb[:, tt, 512:DV])
                    mv = olnp.tile([128, 2], F32, tag="mv")
                    nc.vector.bn_aggr(mv[:], stats[:])
                    std = olnp.tile([128, 1], F32, tag="sd")
                    nc.scalar.activation(std[:], mv[:, 1:2], AF.Sqrt,
                                         bias=eps_sb[:])
                    nc.vector.reciprocal(std[:], std[:])
                    lno = olnp.tile([128, DV], BF16, tag="lno")
                    nc.vector.tensor_scalar(lno[:], out_sb[:, tt, :],
                                            mv[:, 0:1], std[:],
                                            ALU.subtract, ALU.mult)
                    for half in range(2):
                        tp = tp_ps.tile([128, 4, 128], BF16, tag="tp")
                        for cb in range(4):
                            nc.tensor.transpose(
                                tp[:, cb, :],
                                lno[:, (half * 4 + cb) * 128:
                                    (half * 4 + cb + 1) * 128], id_sb[:])
                        nc.scalar.activation(
                            lnoT[:, half * 4:half * 4 + 4,
                                 it * 128:(it + 1) * 128], tp[:], AF.Copy)
                for co in range(D // 128):
                    ps = mm_ps.tile([128, 512], F32, tag="mm")
                    for ci in range(DV // 128):
                        nc.tensor.matmul(ps[:],
                                         outw_sb[:, ci, co * 128:(co + 1) * 128],
                                         lnoT[:, ci, :],
                                         start=(ci == 0),
                                         stop=(ci == DV // 128 - 1))
                    nc.scalar.activation(
                        h2_cm[:, co, CP + q * 512:CP + (q + 1) * 512], ps[:],
                        AF.Silu, bias=outb_sb[:, co:co + 1])

        outsbp_ctx.__exit__(None, None, None)

        # ------------- Stage 7: AllGather conv halo -------------
        ag_in = dram.tile([16, D], F32)
        ag_out = dram.tile([NC * 16, D], F32, addr_space="Shared")
        for co in range(D // 128):
            cs = slice(co * 128, (co + 1) * 128)
            nc.gpsimd.dma_start(ag_in[0:8, cs].rearrange("t c -> c t"),
                                h2_cm[:, co, CP:CP + 8])
            nc.gpsimd.dma_start(ag_in[8:16, cs].rearrange("t c -> c t"),
                                h2_cm[:, co, T:T + CP])
        nc.gpsimd.collective_compute("AllGather", ALU.bypass, replica_groups=RG,
                                     ins=[ag_in[:]], outs=[ag_out[:]])
        with tc.tile_pool(name="agp", bufs=1) as agp:
            ag_sb = agp.tile([128, D], F32)
            nc.sync.dma_start(ag_sb[:], ag_out[:])
            for co in range(D // 128):
                hps = att_ps.tile([128, 512], F32, tag="att")
                nc.tensor.matmul(hps[:, 0:16], ag_sb[:, co * 128:(co + 1) * 128],
                                 sel_sb[:], start=True, stop=True)
                nc.vector.tensor_copy(h2_cm[:, co, 0:CP], hps[:, 0:CP])
                nc.vector.tensor_copy(h2_cm[:, co, T + CP:T + 2 * CP],
                                      hps[:, CP:2 * CP])

        # ------------- Stage 8: final conv + residual + store -------------
        # q-outer / co-inner so interior token slabs (q=1,2) fully complete
        # before the AllGather-dependent edge slabs (q=0,3); transposes ride
        # the PE (packed bf16 PSUM) and the residual add reads PSUM directly.
        with tc.tile_pool(name="fddp", bufs=1) as fddp, \
             tc.tile_pool(name="fcm", bufs=6) as fcmp, \
             tc.tile_pool(name="fin", bufs=3) as finp:
            fdd = fddp.tile([128, D // 128, KER, 128], BF16)
            nc.scalar.dma_start(
                fdd[:], dd[NCH_H + 1:NCH_ALL].rearrange("c k p f -> p c k f"))
            for q in (1, 2, 0, 3):
                hcms = []
                for co in range(D // 128):
                    ps = mm_ps.tile([128, 512], F32, tag="mm")
                    for k in range(KER):
                        nc.tensor.matmul(ps[:], fdd[:, co, k, :],
                                         h2_cm[:, co, q * 512 + k:
                                               q * 512 + k + 512],
                                         start=(k == 0), stop=False)
                    nc.tensor.matmul(ps[:], id_sb[:],
                                     h2_cm[:, co, q * 512 + CP:
                                           q * 512 + CP + 512],
                                     start=False, stop=True)
                    fcm = fcmp.tile([128, 512], F32, tag="fcm")
                    nc.scalar.activation(fcm[:], ps[:], AF.Copy)
                    hcms.append(fcm)
                for b in range(4):
                    tb = q * 4 + b
                    tp = tp_ps.tile([128, 4, 128], F32, tag="tp")
                    for co in range(D // 128):
                        nc.tensor.transpose(
                            tp[:, co, :], hcms[co][:, b * 128:(b + 1) * 128],
                            id32_sb[:])
                    xres = finp.tile([128, D], F32, tag="xr")
                    dma_eng().dma_start(xres[:],
                                        x_sh[129 + tb * 128:257 + tb * 128, :])
                    fin = finp.tile([128, D], F32, tag="fin")
                    nc.vector.tensor_add(fin[:],
                                         tp[:].rearrange("p a b -> p (a b)"),
                                         xres[:])
                    nc.sync.dma_start(y[tb * 128:(tb + 1) * 128, :], fin[:])
        h2p_ctx.__exit__(None, None, None)


_NC_CACHE = None


def _get_nc():
    global _NC_CACHE
    if _NC_CACHE is None:
        _NC_CACHE = _build_kernel()
    return _NC_CACHE


def _prep_inputs(inputs):
    """Host-side preprocessing: LN-affine folds, diag conv matrices,
    per-core shards."""
    g = {k: np.asarray(v) for k, v in inputs.items()}
    x = g['x'].reshape(N, D).astype(np.float32)
    inv_n = np.float32(g['inv_n'])

    thw = (g['th_ln_g'][:, None] * g['th_w']).astype(np.float32)
    thb = (g['th_b'] + g['th_ln_b'] @ g['th_w']).astype(np.float32)
    qkw = (g['qk_ln_g'][:, None] * g['qk_w']).astype(np.float32)
    qkb = (g['qk_b'] + g['qk_ln_b'] @ g['qk_w']).astype(np.float32)
    outw = (g['out_ln_g'][:, None] * g['out_w']).astype(np.float32)
    outb = (g['out_b'] + g['out_ln_b'] @ g['out_w']).astype(np.float32)
    osg = g['os_gamma'].astype(np.float32).copy()
    osb = g['os_beta'].astype(np.float32).copy()
    osg[0] /= GS
    osb[0] /= GS

    # diag conv matrices (identity tap folded in: +I at k=8) + raw tap
    # vectors for the DVE/GpSimd tap share
    ddm = np.zeros((NCH_ALL, KER, 128, 128), np.float32)
    kerv = np.zeros((NCH_ALL, 128, KER), np.float32)
    kers = [g['th_conv'][:, 0, :], g['qk_conv'][:, 0, :], g['out_conv'][:, 0, :]]
    chunk = 0
    for ker in kers:
        C = ker.shape[0]
        for cb in range(C // 128):
            for k in range(KER):
                v = ker[cb * 128:(cb + 1) * 128, k].copy()
                np.fill_diagonal(ddm[chunk, k], v)
                kerv[chunk, :, k] = ker[cb * 128:(cb + 1) * 128, k]
            chunk += 1
    assert chunk == NCH_ALL

    xpad = np.zeros((N + 2 * HALO + 1, D), np.float32)
    xpad[HALO + 1:HALO + 1 + N] = x

    import ml_dtypes
    bf = ml_dtypes.bfloat16
    shared = dict(
        thw=thw.astype(bf), thb=thb,
        qkw=qkw.astype(bf), qkb=qkb,
        outw=outw.astype(bf), outb=outb,
        dd=ddm.astype(bf),
        kerv=kerv,
        ident=np.eye(128, dtype=np.float32).astype(bf),
        osg=osg, osb=osb,
        invn=np.full((128, 1), inv_n, np.float32),
    )

    in_maps = []
    for c in range(NC):
        s = c * T
        me = np.ones((128, 256), np.float32)
        if c == 0:
            me[:, :128] = 0.0
        if c == NC - 1:
            me[:, 128:] = 0.0
        selm = np.zeros((128, 16), np.float32)
        for m in range(8):
            if c > 0:
                selm[(c - 1) * 16 + 8 + m, m] = 1.0
            if c < NC - 1:
                selm[(c + 1) * 16 + m, 8 + m] = 1.0
        im = dict(shared)
        im['x_sh'] = np.ascontiguousarray(xpad[s:s + T2 + 1])
        im['medge'] = me
        im['sel'] = selm
        in_maps.append(im)
    return in_maps


def kernel(**inputs):
    nc = _get_nc()
    in_maps = _prep_inputs(inputs)
    res = run_bass_kernel_spmd(nc, in_maps, core_ids=list(range(NC)))
    out = np.concatenate([r['y'] for r in res.results], axis=0)
    return out.reshape(1, N, D).astype(np.float32)


# revision 49
# speedup vs baseline: 1.5437x; 1.0518x over previous
"""FLASH (ShareA, FFConvM) Trainium2 kernel — 8-core SPMD.

Strategy (per the sharding hint): shard the 16384-token sequence across the
8 NeuronCores (2048 tokens each, group-aligned: 8 groups of 256 per core).
Each core computes both FFConvM branches with a 128-token halo so that the
17-tap depthwise convs match the unsharded reference exactly; the global
linear-attention path AllReduces the [128, 1024] lin_kv / lin_ku summaries;
the final FFConvM's depthwise conv exchanges an 8-token boundary halo via a
small AllGather.

All matmuls run in bf16 on the tensor engine (fp32 matmul is 1/4 rate);
LayerNorm statistics, PSUM accumulation, and the residual path stay fp32.
Depthwise convs are evaluated on the PE as 17 PSUM-accumulated matmuls with
per-tap diagonal weight matrices plus one exact-identity matmul for the
conv residual (keeping 1+k out of bf16). Layout transposes (token-major <->
channel-major) are split between the DMA XBAR transpose engine (sync queue)
and PE transposes evacuated through packed bf16 PSUM banks, so neither
resource serializes the pipeline.
"""
import sys

if '/opt/trn_rl_repo' not in sys.path:
    sys.path.insert(0, '/opt/trn_rl_repo')

import numpy as np

import concourse.bass as bass
import concourse.tile as tile
from concourse import bacc, mybir
from concourse.bass_utils import run_bass_kernel_spmd

F32 = mybir.dt.float32
BF16 = mybir.dt.bfloat16
AF = mybir.ActivationFunctionType
ALU = mybir.AluOpType

N, D, H2, DV, QK, GS, KER = 16384, 512, 2048, 1024, 128, 256, 17
NC = 8
T = N // NC            # 2048 own tokens
HALO = 128
T2 = T + 2 * HALO      # 2304 pre-activation tokens
CP = (KER - 1) // 2    # 8 (conv halo)
RG = [list(range(NC))]

NCH_H = H2 // 128      # 16 chunks for h
NCH_ALL = NCH_H + 1 + D // 128   # 16 h + 1 qk + 4 out = 21
NTT = T // 128         # 16 token tiles of own range
NLT = T2 // 128        # 18 LN tiles


def _build_kernel():
    nc = bacc.Bacc("TRN2", target_bir_lowering=False, debug=False,
                   num_devices=NC)

    # ---------------- I/O ----------------
    x_sh = nc.dram_tensor("x_sh", [T2 + 1, D], F32, kind="ExternalInput").ap()
    thw = nc.dram_tensor("thw", [D, H2], BF16, kind="ExternalInput").ap()
    thb = nc.dram_tensor("thb", [H2], F32, kind="ExternalInput").ap()
    qkw = nc.dram_tensor("qkw", [D, QK], BF16, kind="ExternalInput").ap()
    qkb = nc.dram_tensor("qkb", [QK], F32, kind="ExternalInput").ap()
    outw = nc.dram_tensor("outw", [DV, D], BF16, kind="ExternalInput").ap()
    outb = nc.dram_tensor("outb", [D], F32, kind="ExternalInput").ap()
    dd = nc.dram_tensor("dd", [NCH_ALL, KER, 128, 128], BF16,
                        kind="ExternalInput").ap()
    osg = nc.dram_tensor("osg", [4, QK], F32, kind="ExternalInput").ap()
    osb = nc.dram_tensor("osb", [4, QK], F32, kind="ExternalInput").ap()
    invn = nc.dram_tensor("invn", [128, 1], F32, kind="ExternalInput").ap()
    medge = nc.dram_tensor("medge", [128, 256], F32, kind="ExternalInput").ap()
    sel = nc.dram_tensor("sel", [128, 16], F32, kind="ExternalInput").ap()
    kerv = nc.dram_tensor("kerv", [NCH_ALL, 128, KER], F32,
                          kind="ExternalInput").ap()
    ident = nc.dram_tensor("ident", [128, 128], BF16,
                           kind="ExternalInput").ap()
    y = nc.dram_tensor("y", [T, D], F32, kind="ExternalOutput").ap()

    with tile.TileContext(nc) as tc:
        _emit(nc, tc, x_sh, thw, thb, qkw, qkb, outw, outb, dd, osg, osb,
              invn, medge, sel, kerv, ident, y)
    nc.compile()
    return nc


DVE_TAPS = ()  # DVE scalar_tensor_tensor only has a 1x uop — PE is 3.4x
GPS_TAPS = ()  # faster per tap, so all taps stay on the tensor engine
PE_TAPS = tuple(k for k in range(KER) if k not in DVE_TAPS + GPS_TAPS)


def _emit(nc, tc, x_sh, thw, thb, qkw, qkb, outw, outb, dd, osg, osb,
          invn, medge, sel, kerv, ident, y):
    from contextlib import ExitStack

    # XBAR transposes stay on the SP (sync) queue — the xbar path is tied
    # to it; plain bulk loads alternate between the two HWDGE queues so
    # they don't mode-switch-serialize with the transposes.
    dma_state = [0]

    def dma_eng():
        dma_state[0] ^= 1
        return nc.scalar if dma_state[0] else nc.sync

    def dma_t(out, in_):
        nc.sync.dma_start_transpose(out, in_)

    ctx = ExitStack()
    with ctx:
        consts = ctx.enter_context(tc.tile_pool(name="consts", bufs=1))
        mm_ps = ctx.enter_context(tc.tile_pool(name="mm_ps", bufs=2, space="PSUM"))
        sim_ps = ctx.enter_context(tc.tile_pool(name="sim_ps", bufs=1, space="PSUM"))
        att_ps = ctx.enter_context(tc.tile_pool(name="att_ps", bufs=3, space="PSUM"))
        tp_ps = ctx.enter_context(tc.tile_pool(name="tp_ps", bufs=2, space="PSUM"))
        dram = ctx.enter_context(tc.tile_pool(name="dram", bufs=1, space="DRAM"))

        # ------------- constants to SBUF -------------
        qkw_sb = consts.tile([128, D // 128, QK], BF16)
        nc.sync.dma_start(qkw_sb[:], qkw.rearrange("(o p) f -> p o f", p=128))
        outw_sb = consts.tile([128, DV // 128, D], BF16)
        nc.sync.dma_start(outw_sb[:], outw.rearrange("(o p) f -> p o f", p=128))
        thb_sb = consts.tile([128, H2 // 128], F32)
        nc.sync.dma_start(thb_sb[:], thb.rearrange("(o p) -> p o", p=128))
        qkb_sb = consts.tile([128, 1], F32)
        nc.sync.dma_start(qkb_sb[:], qkb.rearrange("(o p) -> p o", p=128))
        outb_sb = consts.tile([128, D // 128], F32)
        nc.sync.dma_start(outb_sb[:], outb.rearrange("(o p) -> p o", p=128))
        osg_sb = consts.tile([128, 4], F32)
        nc.sync.dma_start(osg_sb[:], osg.rearrange("m p -> p m"))
        osb_sb = consts.tile([128, 4], F32)
        nc.sync.dma_start(osb_sb[:], osb.rearrange("m p -> p m"))
        inv_sb = consts.tile([128, 1], F32)
        nc.sync.dma_start(inv_sb[:], invn)
        me_sb = consts.tile([128, 256], F32)
        nc.sync.dma_start(me_sb[:], medge)
        sel_sb = consts.tile([128, 16], F32)
        nc.sync.dma_start(sel_sb[:], sel)
        kerv_sb = consts.tile([128, NCH_ALL, KER], F32)
        nc.sync.dma_start(kerv_sb[:], kerv.rearrange("c p k -> p c k"))
        id_sb = consts.tile([128, 128], BF16)
        nc.sync.dma_start(id_sb[:], ident)
        id32_sb = consts.tile([128, 128], F32)
        nc.vector.tensor_copy(id32_sb[:], id_sb[:])
        eps_sb = consts.tile([128, 1], F32)
        nc.vector.memset(eps_sb[:], 1e-5)

        # resident activations (whole-kernel lifetime)
        qs_cm = consts.tile([128, 4, T], BF16)       # [d, m, t']  m: qq,lq,qk_,lk
        lin_kT = consts.tile([128, NTT, QK], BF16)   # [t'%128, tt, d]
        vuT = consts.tile([128, NTT, H2], BF16)      # [t'%128, tt, c] (v|u)
        linkvu_bf = consts.tile([128, 2 * DV], BF16)  # [d, (kv|ku)]

        # ------------- Stage 1: LN + transpose -------------
        with tc.tile_pool(name="thwp", bufs=1) as thwp, \
             tc.tile_pool(name="ln", bufs=3) as lnp, \
             tc.tile_pool(name="xnt", bufs=1) as xntp:
            thw_sb = thwp.tile([128, D // 128, H2], BF16)
            nc.sync.dma_start(thw_sb[:],
                              thw.rearrange("(o p) f -> p o f", p=128))
            xnT = xntp.tile([128, D // 128, T2], BF16)   # [ci%128, ci_chunk, j]
            for tt in range(NLT):
                r0 = tt * 128
                nxt = lnp.tile([128, D], F32, tag="nxt")
                dma_eng().dma_start(nxt[:, 0:D // 2], x_sh[r0:r0 + 128, 0:D // 2])
                dma_eng().dma_start(nxt[:, D // 2:D],
                                  x_sh[r0 + 1:r0 + 129, D // 2:D])
                stats = lnp.tile([128, 6], F32, tag="st")
                nc.vector.bn_stats(stats[:], nxt[:])
                mv = lnp.tile([128, 2], F32, tag="mv")
                nc.vector.bn_aggr(mv[:], stats[:])
                std = lnp.tile([128, 1], F32, tag="sd")
                nc.scalar.activation(std[:], mv[:, 1:2], AF.Sqrt, bias=eps_sb[:])
                nc.vector.reciprocal(std[:], std[:])
                xn = lnp.tile([128, D], BF16, tag="xn")
                nc.vector.tensor_scalar(xn[:], nxt[:], mv[:, 0:1], std[:],
                                        ALU.subtract, ALU.mult)
                # transpose on the (otherwise idle) PE; 4 blocks packed into
                # one bf16 PSUM bank, evacuated with a single strided copy
                tp = tp_ps.tile([128, D // 128, 128], BF16, tag="tp")
                for cb in range(D // 128):
                    nc.tensor.transpose(tp[:, cb, :],
                                        xn[:, cb * 128:(cb + 1) * 128], id_sb[:])
                nc.scalar.activation(xnT[:, :, r0:r0 + 128], tp[:], AF.Copy)

            # lin_kv / lin_ku summaries + AllReduce, in halves: the kv
            # half (v = chunks 0-7) fires while the u chunks still compute,
            # hiding most of the collective latency.
            ar_in = [dram.tile([128, DV], BF16, name=f"ari{h}")
                     for h in range(2)]
            ar_out = [dram.tile([128, DV], BF16, addr_space="Shared",
                                name=f"aro{h}") for h in range(2)]

            def emit_summary(half, lkvp):
                lk = lkvp.tile([128, DV], BF16, tag="lk", name=f"lk{half}")
                for e2 in range(2):
                    es = half * 2 + e2
                    ps = mm_ps.tile([128, 512], F32, tag="mm")
                    for tb in range(NTT):
                        nc.tensor.matmul(ps[:], lin_kT[:, tb, :],
                                         vuT[:, tb, es * 512:(es + 1) * 512],
                                         start=(tb == 0), stop=(tb == NTT - 1))
                    nc.vector.tensor_scalar_mul(lk[:, e2 * 512:(e2 + 1) * 512],
                                                ps[:], inv_sb[:, 0:1])
                nc.sync.dma_start(ar_in[half][:], lk[:])
                nc.gpsimd.collective_compute(
                    "AllReduce", ALU.add, replica_groups=RG,
                    ins=[ar_in[half][:]], outs=[ar_out[half][:]])
                nc.sync.dma_start(linkvu_bf[:, half * DV:(half + 1) * DV],
                                  ar_out[half][:])

            # ------------- Stage 2+3: linears + convs  -------------
            with tc.tile_pool(name="pre", bufs=3) as prep, \
                 tc.tile_pool(name="ddp", bufs=2) as ddp, \
                 tc.tile_pool(name="lkv", bufs=2) as lkvp, \
                 tc.tile_pool(name="hcm", bufs=3) as hcmp:
                # order: qk chunk first (frees the qk/attention path early)
                for ch in range(NCH_H + 1):
                    is_qk = (ch == 0)
                    wch = NCH_H if is_qk else ch - 1   # chunk id in dd
                    pre = prep.tile([128, T2], BF16, tag="pre")
                    for r in range(5):
                        j0 = r * 512
                        w = 512 if r < 4 else T2 - 2048
                        ps = mm_ps.tile([128, 512], F32, tag="mm")
                        for ci in range(D // 128):
                            wsrc = (qkw_sb[:, ci, :] if is_qk else
                                    thw_sb[:, ci,
                                           (wch) * 128:(wch + 1) * 128])
                            nc.tensor.matmul(ps[:, :w], wsrc,
                                             xnT[:, ci, j0:j0 + w],
                                             start=(ci == 0),
                                             stop=(ci == D // 128 - 1))
                        bias = (qkb_sb[:, 0:1] if is_qk else
                                thb_sb[:, wch:wch + 1])
                        nc.scalar.activation(pre[:, j0:j0 + w], ps[:, :w],
                                             AF.Silu, bias=bias)
                    # zero tokens outside the global sequence (edge cores)
                    nc.vector.tensor_mul(pre[:, 0:128], pre[:, 0:128],
                                         me_sb[:, 0:128])
                    nc.vector.tensor_mul(pre[:, T2 - 128:T2],
                                         pre[:, T2 - 128:T2],
                                         me_sb[:, 128:256])

                    dk = ddp.tile([128, KER, 128], BF16, tag="dk")
                    dma_eng().dma_start(dk[:], dd[wch].rearrange("k p f -> p k f"))
                    pe_taps = range(KER) if is_qk else PE_TAPS
                    for q in range(4):
                        ps = mm_ps.tile([128, 512], F32, tag="mm")
                        for k in pe_taps:
                            nc.tensor.matmul(
                                ps[:], dk[:, k, :],
                                pre[:, 120 + q * 512 + k:632 + q * 512 + k],
                                start=(k == pe_taps[0] if not is_qk else k == 0),
                                stop=False)
                        # conv residual via an exact bf16 identity matmul
                        nc.tensor.matmul(
                            ps[:], id_sb[:],
                            pre[:, 128 + q * 512:640 + q * 512],
                            start=False, stop=True)
                        if is_qk:
                            for m in range(4):
                                nc.vector.tensor_scalar(
                                    qs_cm[:, m, q * 512:(q + 1) * 512], ps[:],
                                    osg_sb[:, m:m + 1], osb_sb[:, m:m + 1],
                                    ALU.mult, ALU.add)
                        else:
                            hcm = hcmp.tile([128, 512], BF16, tag="hcm")
                            nc.scalar.activation(hcm[:], ps[:], AF.Copy)
                            # transpose: 2 blocks via the XBAR, 2 via the
                            # PE (packed psum + one strided ACT copy) to
                            # split the load between the two resources
                            for b in (0, 2):
                                tb = q * 4 + b
                                dma_t(
                                    vuT[:, tb, (wch) * 128:(wch + 1) * 128],
                                    hcm[:, b * 128:(b + 1) * 128])
                            tp = tp_ps.tile([128, 4, 128], BF16, tag="tp")
                            for i, b in enumerate((1, 3)):
                                nc.tensor.transpose(
                                    tp[:, i, :],
                                    hcm[:, b * 128:(b + 1) * 128], id_sb[:])
                                nc.scalar.activation(
                                    vuT[:, q * 4 + b,
                                        (wch) * 128:(wch + 1) * 128],
                                    tp[:, i, :], AF.Copy)
                    if is_qk:
                        for tb in range(NTT):
                            dma_t(
                                lin_kT[:, tb, :],
                                qs_cm[:, 3, tb * 128:(tb + 1) * 128])
                    elif wch == 7:
                        emit_summary(0, lkvp)
                    elif wch == NCH_H - 1:
                        emit_summary(1, lkvp)

        h2p_ctx = tc.tile_pool(name="h2cm", bufs=1)
        h2p = h2p_ctx.__enter__()
        h2_cm = h2p.tile([128, D // 128, T + 2 * CP], BF16)  # [co, ch, z]
        outsbp_ctx = tc.tile_pool(name="outsb", bufs=1)
        outsbp = outsbp_ctx.__enter__()
        out_sb = outsbp.tile([128, NTT, DV], BF16)  # gated out, token-major

        # ------------- Stage 5: quadratic attention + gating -------------
        with tc.tile_pool(name="attn", bufs=3) as attnp, \
             tc.tile_pool(name="gat", bufs=3) as gatp:
            for g in range(T // GS):
                j0 = g * GS
                rr = attnp.tile([128, 2, GS], BF16, tag="rr")
                for jh in range(2):
                    sps = sim_ps.tile([128, GS], F32, tag="sim")
                    nc.tensor.matmul(
                        sps[:], qs_cm[:, 2, j0 + jh * 128:j0 + (jh + 1) * 128],
                        qs_cm[:, 0, j0:j0 + GS], start=True, stop=True)
                    nc.scalar.activation(rr[:, jh, :], sps[:], AF.Relu)
                at = attnp.tile([128, 2, GS], BF16, tag="at")
                nc.vector.tensor_mul(at[:], rr[:], rr[:])
                for ih in range(2):
                    tt = 2 * g + ih
                    psl = []
                    for es in range(4):
                        ps = att_ps.tile([128, 512], F32, tag="att")
                        psl.append(ps)
                        for jh in range(2):
                            nc.tensor.matmul(
                                ps[:], at[:, jh, ih * 128:(ih + 1) * 128],
                                vuT[:, 2 * g + jh, es * 512:(es + 1) * 512],
                                start=(jh == 0), stop=False)
                        nc.tensor.matmul(
                            ps[:], qs_cm[:, 1, j0 + ih * 128:j0 + (ih + 1) * 128],
                            linkvu_bf[:, es * 512:(es + 1) * 512],
                            start=False, stop=True)
                    # gating: out = att_u * v * sigmoid(att_v * u)
                    t1 = gatp.tile([128, DV], BF16, tag="t1")
                    t2 = gatp.tile([128, DV], BF16, tag="t2")
                    for es in range(2):
                        sl = slice(es * 512, (es + 1) * 512)
                        nc.vector.tensor_mul(t1[:, sl], psl[es][:],
                                             vuT[:, tt, DV + es * 512:
                                                 DV + (es + 1) * 512])
                        nc.vector.tensor_mul(t2[:, sl], psl[es + 2][:],
                                             vuT[:, tt, sl])
                    sg = gatp.tile([128, DV], BF16, tag="sg")
                    nc.scalar.activation(sg[:], t1[:], AF.Sigmoid)
                    nc.vector.tensor_mul(out_sb[:, tt, :], t2[:], sg[:])

        # ------------- Stage 6: out-LN + out-linear -------------
        with tc.tile_pool(name="oln", bufs=3) as olnp, \
             tc.tile_pool(name="lnt", bufs=2) as lntp:
            for q in range(4):
                lnoT = lntp.tile([128, DV // 128, 512], BF16, tag="lnoT")
                for it in range(4):
                    tt = q * 4 + it
                    stats = olnp.tile([128, 2, 6], F32, tag="st")
                    nc.vector.bn_stats(stats[:, 0, :], out_sb[:, tt, 0:512])
                    nc.vector.bn_stats(stats[:, 1, :], out_sb[:, tt, 512:DV])
                    mv = olnp.tile([128, 2], F32, tag="mv")
                    nc.vector.bn_aggr(mv[:], stats[:])
                    std = olnp.tile([128, 1], F32, tag="sd")
                    nc.scalar.activation(std[:], mv[:, 1:2], AF.Sqrt,
                                         bias=eps_sb[:])
                    nc.vector.reciprocal(std[:], std[:])
                    lno = olnp.tile([128, DV], BF16, tag="lno")
                    nc.vector.tensor_scalar(lno[:], out_sb[:, tt, :],
                                            mv[:, 0:1], std[:],
                                            ALU.subtract, ALU.mult)
                    for half in range(2):
                        tp = tp_ps.tile([128, 4, 128], BF16, tag="tp")
                        for cb in range(4):
                            nc.tensor.transpose(
                                tp[:, cb, :],
                                lno[:, (half * 4 + cb) * 128:
                                    (half * 4 + cb + 1) * 128], id_sb[:])
                        nc.scalar.activation(
                            lnoT[:, half * 4:half * 4 + 4,
                                 it * 128:(it + 1) * 128], tp[:], AF.Copy)
                for co in range(D // 128):
                    ps = mm_ps.tile([128, 512], F32, tag="mm")
                    for ci in range(DV // 128):
                        nc.tensor.matmul(ps[:],
                                         outw_sb[:, ci, co * 128:(co + 1) * 128],
                                         lnoT[:, ci, :],
                                         start=(ci == 0),
                                         stop=(ci == DV // 128 - 1))
                    nc.scalar.activation(
                        h2_cm[:, co, CP + q * 512:CP + (q + 1) * 512], ps[:],
                        AF.Silu, bias=outb_sb[:, co:co + 1])

        outsbp_ctx.__exit__(None, None, None)

        # ------------- Stage 7: AllGather conv halo (fires early) -------
        ag_in = dram.tile([16, D], F32)
        ag_out = dram.tile([NC * 16, D], F32, addr_space="Shared")
        for co in range(D // 128):
            cs = slice(co * 128, (co + 1) * 128)
            nc.gpsimd.dma_start(ag_in[0:8, cs].rearrange("t c -> c t"),
                                h2_cm[:, co, CP:CP + 8])
            nc.gpsimd.dma_start(ag_in[8:16, cs].rearrange("t c -> c t"),
                                h2_cm[:, co, T:T + CP])
        nc.gpsimd.collective_compute("AllGather", ALU.bypass, replica_groups=RG,
                                     ins=[ag_in[:]], outs=[ag_out[:]])

        # ------------- Stage 8: final conv + residual + store -------------
        # q-outer / co-inner so interior token slabs (q=1,2) fully complete
        # before the AllGather-dependent edge slabs (q=0,3); transposes ride
        # the PE (packed bf16 PSUM) and the residual add reads PSUM directly.
        with tc.tile_pool(name="fddp", bufs=1) as fddp, \
             tc.tile_pool(name="fcm", bufs=6) as fcmp, \
             tc.tile_pool(name="agp", bufs=1) as agp, \
             tc.tile_pool(name="fin", bufs=3) as finp:
            fdd = fddp.tile([128, D // 128, KER, 128], BF16)
            nc.scalar.dma_start(
                fdd[:], dd[NCH_H + 1:NCH_ALL].rearrange("c k p f -> p c k f"))
            def emit_halo_select():
                ag_sb = agp.tile([128, D], F32)
                nc.sync.dma_start(ag_sb[:], ag_out[:])
                for co in range(D // 128):
                    hps = att_ps.tile([128, 512], F32, tag="att")
                    nc.tensor.matmul(hps[:, 0:16],
                                     ag_sb[:, co * 128:(co + 1) * 128],
                                     sel_sb[:], start=True, stop=True)
                    nc.vector.tensor_copy(h2_cm[:, co, 0:CP], hps[:, 0:CP])
                    nc.vector.tensor_copy(h2_cm[:, co, T + CP:T + 2 * CP],
                                          hps[:, CP:2 * CP])

            for q in (1, 2, 0, 3):
                if q == 0:
                    emit_halo_select()
                hcms = []
                for co in range(D // 128):
                    ps = mm_ps.tile([128, 512], F32, tag="mm")
                    for k in range(KER):
                        nc.tensor.matmul(ps[:], fdd[:, co, k, :],
                                         h2_cm[:, co, q * 512 + k:
                                               q * 512 + k + 512],
                                         start=(k == 0), stop=False)
                    nc.tensor.matmul(ps[:], id_sb[:],
                                     h2_cm[:, co, q * 512 + CP:
                                           q * 512 + CP + 512],
                                     start=False, stop=True)
                    fcm = fcmp.tile([128, 512], F32, tag="fcm")
                    nc.scalar.activation(fcm[:], ps[:], AF.Copy)
                    hcms.append(fcm)
                for b in range(4):
                    tb = q * 4 + b
                    tp = tp_ps.tile([128, 4, 128], F32, tag="tp")
                    for co in range(D // 128):
                        nc.tensor.transpose(
                            tp[:, co, :], hcms[co][:, b * 128:(b + 1) * 128],
                            id32_sb[:])
                    xres = finp.tile([128, D], F32, tag="xr")
                    dma_eng().dma_start(xres[:],
                                        x_sh[129 + tb * 128:257 + tb * 128, :])
                    fin = finp.tile([128, D], F32, tag="fin")
                    nc.vector.tensor_add(fin[:],
                                         tp[:].rearrange("p a b -> p (a b)"),
                                         xres[:])
                    nc.sync.dma_start(y[tb * 128:(tb + 1) * 128, :], fin[:])
        h2p_ctx.__exit__(None, None, None)


_NC_CACHE = None


def _get_nc():
    global _NC_CACHE
    if _NC_CACHE is None:
        _NC_CACHE = _build_kernel()
    return _NC_CACHE


def _prep_inputs(inputs):
    """Host-side preprocessing: LN-affine folds, diag conv matrices,
    per-core shards."""
    g = {k: np.asarray(v) for k, v in inputs.items()}
    x = g['x'].reshape(N, D).astype(np.float32)
    inv_n = np.float32(g['inv_n'])

    thw = (g['th_ln_g'][:, None] * g['th_w']).astype(np.float32)
    thb = (g['th_b'] + g['th_ln_b'] @ g['th_w']).astype(np.float32)
    qkw = (g['qk_ln_g'][:, None] * g['qk_w']).astype(np.float32)
    qkb = (g['qk_b'] + g['qk_ln_b'] @ g['qk_w']).astype(np.float32)
    outw = (g['out_ln_g'][:, None] * g['out_w']).astype(np.float32)
    outb = (g['out_b'] + g['out_ln_b'] @ g['out_w']).astype(np.float32)
    osg = g['os_gamma'].astype(np.float32).copy()
    osb = g['os_beta'].astype(np.float32).copy()
    osg[0] /= GS
    osb[0] /= GS

    # diag conv matrices (identity tap folded in: +I at k=8) + raw tap
    # vectors for the DVE/GpSimd tap share
    ddm = np.zeros((NCH_ALL, KER, 128, 128), np.float32)
    kerv = np.zeros((NCH_ALL, 128, KER), np.float32)
    kers = [g['th_conv'][:, 0, :], g['qk_conv'][:, 0, :], g['out_conv'][:, 0, :]]
    chunk = 0
    for ker in kers:
        C = ker.shape[0]
        for cb in range(C // 128):
            for k in range(KER):
                v = ker[cb * 128:(cb + 1) * 128, k].copy()
                np.fill_diagonal(ddm[chunk, k], v)
                kerv[chunk, :, k] = ker[cb * 128:(cb + 1) * 128, k]
            chunk += 1
    assert chunk == NCH_ALL

    xpad = np.zeros((N + 2 * HALO + 1, D), np.float32)
    xpad[HALO + 1:HALO + 1 + N] = x

    import ml_dtypes
    bf = ml_dtypes.bfloat16
    shared = dict(
        thw=thw.astype(bf), thb=thb,
        qkw=qkw.astype(bf), qkb=qkb,
        outw=outw.astype(bf), outb=outb,
        dd=ddm.astype(bf),
        kerv=kerv,
        ident=np.eye(128, dtype=np.float32).astype(bf),
        osg=osg, osb=osb,
        invn=np.full((128, 1), inv_n, np.float32),
    )

    in_maps = []
    for c in range(NC):
        s = c * T
        me = np.ones((128, 256), np.float32)
        if c == 0:
            me[:, :128] = 0.0
        if c == NC - 1:
            me[:, 128:] = 0.0
        selm = np.zeros((128, 16), np.float32)
        for m in range(8):
            if c > 0:
                selm[(c - 1) * 16 + 8 + m, m] = 1.0
            if c < NC - 1:
                selm[(c + 1) * 16 + m, 8 + m] = 1.0
        im = dict(shared)
        im['x_sh'] = np.ascontiguousarray(xpad[s:s + T2 + 1])
        im['medge'] = me
        im['sel'] = selm
        in_maps.append(im)
    return in_maps


def kernel(**inputs):
    nc = _get_nc()
    in_maps = _prep_inputs(inputs)
    res = run_bass_kernel_spmd(nc, in_maps, core_ids=list(range(NC)))
    out = np.concatenate([r['y'] for r in res.results], axis=0)
    return out.reshape(1, N, D).astype(np.float32)
